# revision 1
# baseline (speedup 1.0000x reference)
"""Trainium2 Bass kernel for nn_JointModalityAttention.

3-modality joint attention, B=8, N=512, D=512, 8 heads x 64.
Sharding: data-parallel over batch -- each of the 8 NeuronCores handles one
batch element; weights replicated; no collectives.

v2 design (vs v1 f32r baseline):
  - Query compaction to n_c=275 (unmasked rows + 1 dummy zero row whose
    uniform-attention output serves every masked row).
  - All projections + AV + out-projection in fp16 (same PE rate as f32r,
    half the DMA bytes).
  - dots in fp8e4 DoubleRow: qT/kT are built in a "folded" layout
    [32 partitions, 2, n] (dh = p + 32j) so one DR matmul contracts the
    full 64-dim head axis at 0.5 cycles/row.  The fold comes free by
    permuting W's columns host-side; q/k are scaled x16 into fp8's sweet
    range and the 1/256 is folded into the exp scale.
  - exp on ScalarE reads the whole 3-chunk PSUM group in one instruction,
    writes fp16 ex tiles; denominator via the ones-column of V_ext
    (row 64 of the AV accumulation).
  - PE program order interleaves mod i+1 projections / mod i-1 out-
    projections into attention heads so the PE fills ScalarE-bound gaps.
"""

import sys

if "/opt/trn_rl_repo" not in sys.path:
    sys.path.insert(0, "/opt/trn_rl_repo")

import ml_dtypes
import numpy as np

import concourse.bass as bass  # noqa: F401  (import keeps bass registered)
import concourse.mybir as mybir
import concourse.tile as tile
from concourse import bacc, bass_utils

HEADS = 8
DH = 64
DI = HEADS * DH  # 512
B = 8
N = 512
D = 512
SCALE = DH ** -0.5
N_CORES = 8
QK_SCALE = 16.0  # q/k pre-scale into fp8 range; folded out in exp scale

F32 = mybir.dt.float32
F16 = mybir.dt.float16
FP8 = mybir.dt.float8e4
DR = mybir.MatmulPerfMode.DoubleRow
NP_F16 = np.float16
NP_FP8 = ml_dtypes.float8_e4m3


def qk_pass_channels(base):
    """Channel order for the 4 folded QK projection passes (g, j).

    Pass (g, j) covers heads 4g..4g+3; PSUM partition p holds channel
    base + head(g,p)*64 + (p%32) + 32*j, so the SBUF tile [128, 2, n]
    (partition p, free j) is exactly the DR lhsT/rhs fold (dh = p%32+32j,
    head = p//32 within group).
    """
    cols = []
    for g in range(2):
        for j in range(2):
            for p in range(128):
                head = 4 * g + p // 32
                dh = (p % 32) + 32 * j
                cols.append(base + head * 64 + dh)
    return cols


def w_perm_cols():
    """Column permutation for the device Wqkv tensor: 4 K passes, 4 Q
    passes (128 cols each), then V natural."""
    return qk_pass_channels(DI) + qk_pass_channels(0) + list(range(2 * DI, 3 * DI))


W_PERM = np.array(w_perm_cols())


def _emit_body(nc, tc, dio, pools, n_c):
    p_w, p_x, p_xq, p_kq, p_v, p_ex, p_oT, p_wo, p_ob, p_den, psA, psB = pools
    Exp = mybir.ActivationFunctionType.Exp
    mm = nc.tensor.matmul

    w, xt, xqt, wo = {}, {}, {}, {}
    kF, qF, V, oT = {}, {}, {}, {}

    xt8, wk8 = {}, {}

    def alloc_inputs(i):
        w[i] = p_w.tile([128, 4, 1536], F16, tag=f"w{i}", name=f"w{i}")
        xt[i] = p_x.tile([128, 4, 512], F16, tag=f"xt{i}", name=f"xt{i}")
        xt8[i] = p_x.tile([128, 4, 512], FP8, tag=f"xt8{i}", name=f"xt8{i}")
        wk8[i] = p_x.tile([128, 4, 512], FP8, tag=f"wk8{i}", name=f"wk8{i}")
        xqt[i] = p_xq.tile([128, 4, n_c], F16, tag=f"xqt{i}", name=f"xqt{i}")

    # one DMA instruction per logical block: each instruction costs a fixed
    # ~625 ns on the serial HWDGE device, so instruction count is what matters
    def dma_w(i, c0, c1):
        src = dio[f"W{i}"].rearrange("(a p) c -> p a c", p=128)
        nc.sync.dma_start(w[i][:, :, c0:c1], src[:, :, c0:c1])

    def dma_xt(i):
        src = dio[f"xT{i}"].rearrange("(a p) c -> p a c", p=128)
        nc.sync.dma_start(xt[i][:, :, :], src)

    def dma_xt8(i):
        src8 = dio[f"xT8{i}"].rearrange("(a p) c -> p a c", p=128)
        nc.sync.dma_start(xt8[i][:, :, :], src8)

    def dma_wk8(i, c0, c1):
        srcw = dio[f"Wk8{i}"].rearrange("(a p) c -> p a c", p=128)
        nc.sync.dma_start(wk8[i][:, :, c0:c1], srcw[:, :, c0:c1])

    def dma_xqt(i):
        src = dio[f"xqT{i}"].rearrange("(a p) c -> p a c", p=128)
        nc.sync.dma_start(xqt[i][:, :, :], src)

    def load_wo(i):
        wo[i] = p_wo.tile([128, 4, 512], F16, tag="wo", name=f"wo{i}")
        src = dio[f"Wout{i}"].rearrange("(a p) c -> p a c", p=128)
        nc.sync.dma_start(wo[i][:, :, :], src)

    def alloc_mod(i):
        kF[i] = [
            p_kq.tile([128, 2, 512], FP8, tag=f"kF{i}{g}", name=f"kF{i}{g}")
            for g in range(2)
        ]
        qF[i] = [
            p_kq.tile([128, 2, n_c], FP8, tag=f"qF{i}{g}", name=f"qF{i}{g}")
            for g in range(2)
        ]
        V[i] = p_v.tile([128, 4, 8, 65], F16, tag=f"V{i}", name=f"V{i}")
        nc.vector.memset(V[i][:, :, :, 64:65], 1.0)
        oT[i] = p_oT.tile([128, 4, n_c], F16, tag=f"oT{i}", name=f"oT{i}")

    def proj_ps(kind, name):
        # "A": prologue passes double-buffer through the idle psA pool;
        # "B": the psB "mm" slot; "C": the psB "avo" slot (free until the
        # first AV accumulation opens) -- B/C alternation double-buffers
        # the start-region passes.
        if kind == "A":
            t = psA.tile([128, 3, 512], F32, tag="dots", name=name)
            return t[:, 0, :]
        tag = "avo" if kind == "C" else "mm"
        return psB.tile([128, 512], F32, tag=tag, name=name)

    def k_pass(i, g, j, pk="B"):
        # fp8 DoubleRow: contraction over d pairs kt-chunks; Wk8 is sent
        # pre-scaled x32 so kF = 0.5*psum carries the same x16 as qF.
        # Split into column halves so the first psum->SBUF copy overlaps
        # the second half's matmuls (the "mm" slot is single-buffered).
        ps = proj_ps(pk, f"psk{i}{g}{j}")
        wsl = wk8[i][:, :, (2 * g + j) * 128 : (2 * g + j + 1) * 128]
        for half in range(2):
            sl = slice(half * 256, (half + 1) * 256)
            for kp in range(2):
                mm(ps[:, sl],
                   wsl[:, 2 * kp : 2 * kp + 2, :],
                   xt8[i][:, 2 * kp : 2 * kp + 2, sl],
                   start=kp == 0, stop=kp == 1, perf_mode=DR)
            nc.vector.tensor_scalar_mul(kF[i][g][:, j, sl], ps[:, sl], 0.5)

    def q_pass(i, g, j, pk="B", act_copy=False):
        # act_copy: route the odd half's psum->fp8 copy through ScalarE
        # (idle in the prologue) so DVE isn't the pass serializer.
        ps = proj_ps(pk, f"psq{i}{g}{j}")
        wsl = w[i][:, :, 512 + (2 * g + j) * 128 : 512 + (2 * g + j + 1) * 128]
        h1 = n_c // 2
        for half in range(2):
            sl = slice(half * h1, (half + 1) * h1 if half == 0 else n_c)
            for kt in range(4):
                mm(ps[:, sl], wsl[:, kt, :], xqt[i][:, kt, sl],
                   start=kt == 0, stop=kt == 3)
            if act_copy and half == 1:
                nc.scalar.mul(qF[i][g][:, j, sl], ps[:, sl], QK_SCALE)
            else:
                nc.vector.tensor_scalar_mul(qF[i][g][:, j, sl], ps[:, sl], QK_SCALE)

    def v_pass(i, mch, pk="B"):
        ps = proj_ps(pk, f"psv{i}{mch}")
        for half in range(2):
            sl = slice(half * 256, (half + 1) * 256)
            for kt in range(4):
                mm(ps[:, sl], xt[i][:, kt, mch * 128 : (mch + 1) * 128],
                   w[i][:, kt, 1024 + half * 256 : 1280 + half * 256],
                   start=kt == 0, stop=kt == 3)
            nc.vector.tensor_copy(
                V[i][:, mch, half * 4 : (half + 1) * 4, 0:64],
                ps[:, sl].rearrange("p (h x) -> p h x", x=64),
            )

    # Attention is split into a dots/exp part and a deferred AV part: the
    # dots of head h+1 are emitted before the AV of head h, so the in-order
    # PE keeps ScalarE fed while the AV accumulator slot ("avo", single
    # buffered) drains through DVE/Pool.
    chunks = [(j_, c) for j_ in range(3) for c in range(4)]
    pending_ex = {}
    pending_pso = {}

    def head_dots(i, h, between=None):
        g, hh = h // 4, h % 4
        kslc = lambda j_, c: kF[j_][g][hh * 32 : hh * 32 + 32, :, c * 128 : (c + 1) * 128]
        qslc = qF[i][g][hh * 32 : hh * 32 + 32, :, :]
        exs = []
        pending_ex[(i, h)] = exs  # filled in place; av parts may read early
        for t in range(4):
            ps_d = psA.tile([128, 3, 512], F32, tag="dots", name=f"psd{i}{h}{t}")
            for u in range(3):
                j_, c = chunks[t * 3 + u]
                mm(ps_d[:, u, 0:n_c], kslc(j_, c), qslc,
                   start=True, stop=True, perf_mode=DR,
                   tile_position=(hh * 32, 0))
            if between is not None and t < len(between) and between[t] is not None:
                for fn in between[t]:
                    fn()
            ex = p_ex.tile([128, 3, n_c], F16, tag="ex", name=f"ex{i}{h}{t}")
            nc.scalar.activation(
                ex[:, :, :], ps_d[:, :, 0:n_c], Exp, scale=SCALE / (QK_SCALE**2)
            )
            exs.append(ex)

    def av_part(i, h, j_):
        # one modality's worth (4 chunks) of the AV accumulation
        if j_ == 0:
            pending_pso[(i, h)] = psB.tile(
                [128, 512], F32, tag="avo", name=f"pso{i}{h}"
            )
        ps_o = pending_pso[(i, h)]
        exs = pending_ex[(i, h)]
        for c in range(4):
            t, u = divmod(j_ * 4 + c, 3)
            mm(ps_o[0:65, 0:n_c], V[j_][:, c, h, :], exs[t][:, u, :],
               start=j_ == 0 and c == 0, stop=j_ == 2 and c == 3)

    def av_drain(i, h):
        ps_o = pending_pso.pop((i, h))
        pending_ex.pop((i, h))
        den_s = p_den.tile([1, 512], F32, tag="dens", name=f"ds{i}{h}")
        nc.vector.reciprocal(den_s[0:1, 0:n_c], ps_o[64:65, 0:n_c])
        den_r = p_den.tile([64, 512], F32, tag="denr", name=f"dr{i}{h}")
        nc.gpsimd.partition_broadcast(den_r[:, 0:n_c], den_s[0:1, 0:n_c])
        po, fh = (h % 2) * 64, h // 2
        nc.vector.tensor_mul(
            oT[i][po : po + 64, fh, :], ps_o[0:64, 0:n_c], den_r[:, 0:n_c]
        )

    def head_av(i, h):
        for j_ in range(3):
            av_part(i, h, j_)
        av_drain(i, h)

    def outproj_chunk(i, s, nn, pk="B"):
        ps = proj_ps(pk, f"psf{i}{s}")
        ob = p_ob.tile([128, 512], F32, tag="ob", name=f"ob{i}{s}")
        for half in range(2):
            sl = slice(half * 256, (half + 1) * 256)
            for ct in range(4):
                mm(ps[0:nn, sl], oT[i][:, ct, s : s + nn], wo[i][:, ct, sl],
                   start=ct == 0, stop=ct == 3)
            nc.vector.tensor_copy(ob[0:nn, sl], ps[0:nn, sl])
        nc.sync.dma_start(dio[f"out{i}"][s : s + nn, :], ob[0:nn, :])

    # ---------------- schedule ----------------
    # DMA emission order = need order: K-g0 cols of all mods first (the
    # prologue), then Q0-g0/V0, V1/V2 (extras of head (0,0)), K-g1
    # (extras (0,1)-(0,3)), then the rest.
    for i in range(3):
        alloc_inputs(i)
    dma_wk8(0, 0, 256)   # K g0 operands (fp8 DR) for the prologue
    dma_xt8(0)
    dma_w(0, 512, 768)   # Q0 g0
    dma_xqt(0)
    dma_wk8(1, 0, 256)
    dma_xt8(1)
    dma_wk8(2, 0, 256)
    dma_xt8(2)
    dma_xt(0)            # V operands (f16 x + W V cols)
    dma_w(0, 1024, 1536)
    dma_xt(1)
    dma_w(1, 1024, 1536)
    dma_xt(2)
    dma_w(2, 1024, 1536)
    dma_w(1, 512, 1024)  # Q1 (needed at the (0,3)->(1,0) boundary)
    dma_xqt(1)
    for i in range(3):   # K g1
        dma_wk8(i, 256, 512)
    dma_w(0, 768, 1024)
    dma_w(2, 512, 1024)
    dma_xqt(2)
    load_wo(0)
    load_wo(1)
    for i in range(3):
        alloc_mod(i)

    # PE warm-up: dependency-free matmuls burn the p-state ramp while the
    # first DMAs land, so the projection passes run at full clock.
    warm = p_den.tile([128, 256], F16, tag="warm", name="warm")
    nc.vector.memset(warm[:, :], 0.0)
    ps_warm = psB.tile([128, 512], F32, tag="mm", name="ps_warm")
    for r in range(90):
        mm(ps_warm[:, 0:64], warm[:, 0:128], warm[:, 0:64],
           start=True, stop=True)

    # Start region: dots of heads (0,0)-(0,3) go out as early as possible;
    # K passes of mods 1/2 and all V passes are woven into the between-
    # group slots (PE work there does not delay ScalarE, which only needs
    # the dots).  The first AV is split per modality behind the V passes.
    for j in range(2):
        k_pass(0, 0, j, pk="A")
    for j in range(2):
        q_pass(0, 0, j, pk="A")
    head_dots(0, 0, between=[
        [lambda: k_pass(1, 0, 0, pk="B"), lambda: k_pass(1, 0, 1, pk="C")],
        [lambda: k_pass(2, 0, 0, pk="B"), lambda: k_pass(2, 0, 1, pk="C")],
        None, None,
    ])
    head_dots(0, 1, between=[
        [lambda: v_pass(0, 0, pk="B")], [lambda: v_pass(0, 1, pk="C")],
        [lambda: v_pass(0, 2, pk="B")], [lambda: v_pass(0, 3, pk="C")],
    ])
    head_dots(0, 2, between=[
        [lambda: v_pass(1, 0, pk="B")], [lambda: v_pass(1, 1, pk="C")],
        [lambda: v_pass(1, 2, pk="B")], [lambda: v_pass(1, 3, pk="C")],
    ])
    head_dots(0, 3, between=[
        [lambda: v_pass(2, 0, pk="B")], [lambda: v_pass(2, 1, pk="C")],
        [lambda: v_pass(2, 2, pk="B")], [lambda: v_pass(2, 3, pk="C")],
    ])
    q_pass(1, 0, 0, pk="B")  # Q1 g0: needed by dots(1,0), the next window
    q_pass(1, 0, 1, pk="C")
    av_part(0, 0, 0)
    av_part(0, 0, 1)
    av_part(0, 0, 2)
    av_drain(0, 0)

    # Remaining head order: all group-0 heads of the 3 mods, then all
    # group-1 heads -- spreads the g1 projection passes thinly over many
    # windows so ScalarE stays fed.  Lag-1 AV deferral via pending queue.
    seq = [
        (i, h)
        for g_ in range(2)
        for i in range(3)
        for h in range(4 * g_, 4 * g_ + 4)
    ][4:]  # (0,0)-(0,3) already emitted above

    # extra PE work per window; every dependency lands >= 1 window early:
    #   Q1g0 before dots(1,0);  Q2g0 before dots(2,0);
    #   K g1 (all mods) + Q0g1 before dots(0,4);  Q1g1 before (1,4);
    #   Q2g1 before (2,4);  outproj(i) after drain(i,7) [lag-1].
    nch_list = [(s, min(128, n_c - s)) for s in range(0, n_c, 128)]
    extras = {
        (1, 0): [lambda j=j: q_pass(2, 0, j) for j in range(2)],
        (1, 2): [lambda j=j: k_pass(0, 1, j) for j in range(2)],
        (1, 3): [lambda j=j: k_pass(1, 1, j) for j in range(2)],
        (2, 0): [lambda j=j: k_pass(2, 1, j) for j in range(2)],
        (2, 1): [lambda j=j: q_pass(0, 1, j) for j in range(2)],
        (2, 2): [lambda j=j: q_pass(1, 1, j) for j in range(2)],
        (2, 3): [lambda j=j: q_pass(2, 1, j) for j in range(2)],
        (2, 4): [lambda: load_wo(2)],
    }
    for k_, (s, nn) in enumerate(nch_list):
        extras[(1, 5 + k_)] = [lambda i=0, s=s, nn=nn: outproj_chunk(i, s, nn)]
        extras[(2, 5 + k_)] = [lambda i=1, s=s, nn=nn: outproj_chunk(i, s, nn)]

    pending = [(0, 1), (0, 2), (0, 3)]
    for i, h in seq[:-1]:
        head_dots(i, h)
        for fn in extras.get((i, h), []):
            fn()
        pending.append((i, h))
        head_av(*pending.pop(0))
        if (i, h) in ((1, 1), (1, 2)) and len(pending) > 1:
            head_av(*pending.pop(0))  # catch up to lag-1 in light windows
    # Last head: the previous head's AV and its own AV parts are woven into
    # its dots groups so the post-exp tail is just the final 4 AV matmuls +
    # drain + out-proj.
    head_dots(2, 7, between=[
        [lambda: head_av(*pending.pop(0))],
        [lambda: head_av(*pending.pop(0))] if len(pending) > 1 else None,
        [lambda: av_part(2, 7, 0)],
        [lambda: av_part(2, 7, 1)],
    ])
    for fn in extras.get((2, 7), []):
        fn()
    av_part(2, 7, 2)
    av_drain(2, 7)
    for s, nn in nch_list:
        outproj_chunk(2, s, nn, pk="A")


def build(n_c=275, reps=1):
    nc = bacc.Bacc("TRN2", target_bir_lowering=False, debug=False)
    dio = {}
    for i in range(3):
        dio[f"xT{i}"] = nc.dram_tensor(f"xT{i}", [D, N], F16, kind="ExternalInput").ap()
        dio[f"xT8{i}"] = nc.dram_tensor(
            f"xT8{i}", [D, N], FP8, kind="ExternalInput"
        ).ap()
        dio[f"Wk8{i}"] = nc.dram_tensor(
            f"Wk8{i}", [D, DI], FP8, kind="ExternalInput"
        ).ap()
        dio[f"xqT{i}"] = nc.dram_tensor(
            f"xqT{i}", [D, n_c], F16, kind="ExternalInput"
        ).ap()
        dio[f"W{i}"] = nc.dram_tensor(
            f"W{i}", [D, 3 * DI], F16, kind="ExternalInput"
        ).ap()
        dio[f"Wout{i}"] = nc.dram_tensor(
            f"Wout{i}", [DI, D], F16, kind="ExternalInput"
        ).ap()
        dio[f"out{i}"] = nc.dram_tensor(
            f"out{i}", [n_c, D], F32, kind="ExternalOutput"
        ).ap()
    with tile.TileContext(nc) as tc:
        with (
            tc.tile_pool(name="w", bufs=1) as p_w,
            tc.tile_pool(name="x", bufs=1) as p_x,
            tc.tile_pool(name="xq", bufs=1) as p_xq,
            tc.tile_pool(name="kq", bufs=1) as p_kq,
            tc.tile_pool(name="v", bufs=1) as p_v,
            tc.tile_pool(name="ex", bufs=24) as p_ex,
            tc.tile_pool(name="oT", bufs=1) as p_oT,
            tc.tile_pool(name="wo", bufs=3) as p_wo,
            tc.tile_pool(name="ob", bufs=4) as p_ob,
            tc.tile_pool(name="den", bufs=4) as p_den,
            tc.tile_pool(name="psA", bufs=2, space="PSUM") as psA,
            tc.tile_pool(name="psB", bufs=1, space="PSUM") as psB,
        ):
            pools = (
                p_w, p_x, p_xq, p_kq, p_v, p_ex, p_oT, p_wo, p_ob, p_den, psA, psB
            )
            for _ in range(reps):
                _emit_body(nc, tc, dio, pools, n_c)
    nc.compile()
    return nc


_BUILD_CACHE = {}


def _get_built(n_c, reps):
    key = (n_c, reps)
    if key not in _BUILD_CACHE:
        _BUILD_CACHE[key] = build(n_c, reps)
    return _BUILD_CACHE[key]


def pick_n_c(inputs):
    """Smallest compacted-query count: unmasked rows + 1 dummy zero row."""
    need = 0
    for i in range(3):
        m = np.asarray(inputs[f"m{i}"]).astype(bool)
        for b in range(B):
            n_u = int(m[b].sum())
            need = max(need, n_u + (1 if n_u < N else 0))
    return max(need, 192)


WK8_SCALE = 32.0  # Wk pre-scale into fp8 range; kF copy multiplies by
                  # QK_SCALE/WK8_SCALE so kF carries the same x16 as qF


def make_in_maps(inputs, n_c=275):
    """Per-core input dicts: transposed fp16 x (+fp8 copy for the K
    DoubleRow projection), compacted fp16 xq, column-permuted fp16 Wqkv,
    fp8 x32 K-weights, fp16 Wout."""
    xs = [np.asarray(inputs[f"x{i}"], dtype=np.float32) for i in range(3)]
    ms = [np.asarray(inputs[f"m{i}"]).astype(bool) for i in range(3)]
    Wp = [
        np.asarray(inputs[f"Wqkv{i}"], np.float32)[:, W_PERM].astype(NP_F16)
        for i in range(3)
    ]
    Wk8 = [
        np.ascontiguousarray(
            WK8_SCALE
            * np.asarray(inputs[f"Wqkv{i}"], np.float32)[:, W_PERM[0:DI]]
        ).astype(NP_FP8)
        for i in range(3)
    ]
    Wo = [np.asarray(inputs[f"Wout{i}"], np.float32).astype(NP_F16) for i in range(3)]
    in_maps = []
    for b in range(B):
        m = {}
        for i in range(3):
            xb = xs[i][b]
            xbT = np.ascontiguousarray(xb.T)
            m[f"xT{i}"] = xbT.astype(NP_F16)
            m[f"xT8{i}"] = xbT.astype(NP_FP8)
            sel = np.flatnonzero(ms[i][b])
            xq = np.zeros((n_c, D), np.float32)
            xq[: len(sel)] = xb[sel[:n_c]]
            m[f"xqT{i}"] = np.ascontiguousarray(xq.T.astype(NP_F16))
            m[f"W{i}"] = Wp[i]
            m[f"Wk8{i}"] = Wk8[i]
            m[f"Wout{i}"] = Wo[i]
        in_maps.append(m)
    return in_maps


def scatter_outputs(results, inputs, n_c):
    ms = [np.asarray(inputs[f"m{i}"]).astype(bool) for i in range(3)]
    outs = []
    for i in range(3):
        full = np.empty((B, N, D), np.float32)
        for b in range(B):
            comp = np.asarray(results[b][f"out{i}"], np.float32)
            sel = np.flatnonzero(ms[i][b])
            full[b][sel] = comp[: len(sel)]
            if len(sel) < N:
                full[b][~ms[i][b]] = comp[len(sel)]
        outs.append(full)
    return outs


def kernel(**inputs):
    n_c = pick_n_c(inputs)
    in_maps = make_in_maps(inputs, n_c)
    nc = _get_built(n_c=n_c, reps=1)
    res = bass_utils.run_bass_kernel_spmd(nc, in_maps, core_ids=list(range(N_CORES)))
    return tuple(scatter_outputs(res.results, inputs, n_c))



# revision 9
# speedup vs baseline: 1.0673x; 1.0673x over previous
"""Trainium2 Bass kernel for nn_JointModalityAttention.

3-modality joint attention, B=8, N=512, D=512, 8 heads x 64.
Sharding: data-parallel over batch -- each of the 8 NeuronCores handles one
batch element; weights replicated; no collectives.

v2 design (vs v1 f32r baseline):
  - Query compaction to n_c=275 (unmasked rows + 1 dummy zero row whose
    uniform-attention output serves every masked row).
  - All projections + AV + out-projection in fp16 (same PE rate as f32r,
    half the DMA bytes).
  - dots in fp8e4 DoubleRow: qT/kT are built in a "folded" layout
    [32 partitions, 2, n] (dh = p + 32j) so one DR matmul contracts the
    full 64-dim head axis at 0.5 cycles/row.  The fold comes free by
    permuting W's columns host-side; q/k are scaled x16 into fp8's sweet
    range and the 1/256 is folded into the exp scale.
  - exp on ScalarE reads the whole 3-chunk PSUM group in one instruction,
    writes fp16 ex tiles; denominator via the ones-column of V_ext
    (row 64 of the AV accumulation).
  - PE program order interleaves mod i+1 projections / mod i-1 out-
    projections into attention heads so the PE fills ScalarE-bound gaps.
"""

import sys

if "/opt/trn_rl_repo" not in sys.path:
    sys.path.insert(0, "/opt/trn_rl_repo")

import ml_dtypes
import numpy as np

import concourse.bass as bass  # noqa: F401  (import keeps bass registered)
import concourse.mybir as mybir
import concourse.tile as tile
from concourse import bacc, bass_utils

HEADS = 8
DH = 64
DI = HEADS * DH  # 512
B = 8
N = 512
D = 512
SCALE = DH ** -0.5
N_CORES = 8
QK_SCALE = 16.0  # q/k pre-scale into fp8 range; folded out in exp scale

F32 = mybir.dt.float32
F16 = mybir.dt.float16
FP8 = mybir.dt.float8e4
DR = mybir.MatmulPerfMode.DoubleRow
NP_F16 = np.float16
NP_FP8 = ml_dtypes.float8_e4m3


def qk_pass_channels(base):
    """Channel order for the 4 folded QK projection passes (g, j).

    Pass (g, j) covers heads 4g..4g+3; PSUM partition p holds channel
    base + head(g,p)*64 + (p%32) + 32*j, so the SBUF tile [128, 2, n]
    (partition p, free j) is exactly the DR lhsT/rhs fold (dh = p%32+32j,
    head = p//32 within group).
    """
    cols = []
    for g in range(2):
        for j in range(2):
            for p in range(128):
                head = 4 * g + p // 32
                dh = (p % 32) + 32 * j
                cols.append(base + head * 64 + dh)
    return cols


def w_perm_cols():
    """Column permutation for the device Wqkv tensor: 4 K passes, 4 Q
    passes (128 cols each), then V natural."""
    return qk_pass_channels(DI) + qk_pass_channels(0) + list(range(2 * DI, 3 * DI))


W_PERM = np.array(w_perm_cols())


def _emit_body(nc, tc, dio, pools, n_c):
    (p_w, p_x, p_xq, p_kq, p_v, p_ex, p_oT, p_wo, p_ob, p_oc,
     p_den, psA, psB) = pools
    Exp = mybir.ActivationFunctionType.Exp
    mm = nc.tensor.matmul

    w, xt, xqt, wo = {}, {}, {}, {}
    kF, qF, V, oT = {}, {}, {}, {}

    xt8, wk8 = {}, {}

    def alloc_inputs(i):
        w[i] = p_w.tile([128, 4, 1536], F16, tag=f"w{i}", name=f"w{i}")
        xt[i] = p_x.tile([128, 4, 512], F16, tag=f"xt{i}", name=f"xt{i}")
        xt8[i] = p_x.tile([128, 4, 512], FP8, tag=f"xt8{i}", name=f"xt8{i}")
        wk8[i] = p_x.tile([128, 4, 512], FP8, tag=f"wk8{i}", name=f"wk8{i}")
        xqt[i] = p_xq.tile([128, 4, n_c], F16, tag=f"xqt{i}", name=f"xqt{i}")

    # one DMA instruction per logical block: each instruction costs a fixed
    # ~625 ns on the serial HWDGE device, so instruction count is what matters
    def dma_w(i, c0, c1):
        src = dio[f"W{i}"].rearrange("(a p) c -> p a c", p=128)
        nc.sync.dma_start(w[i][:, :, c0:c1], src[:, :, c0:c1])

    def dma_xt(i):
        src = dio[f"xT{i}"].rearrange("(a p) c -> p a c", p=128)
        nc.sync.dma_start(xt[i][:, :, :], src)

    def dma_xt8(i):
        src8 = dio[f"xT8{i}"].rearrange("(a p) c -> p a c", p=128)
        nc.sync.dma_start(xt8[i][:, :, :], src8)

    def dma_wk8(i, c0, c1):
        srcw = dio[f"Wk8{i}"].rearrange("(a p) c -> p a c", p=128)
        nc.sync.dma_start(wk8[i][:, :, c0:c1], srcw[:, :, c0:c1])

    def dma_xqt(i):
        src = dio[f"xqT{i}"].rearrange("(a p) c -> p a c", p=128)
        nc.sync.dma_start(xqt[i][:, :, :], src)

    def load_wo(i):
        wo[i] = p_wo.tile([128, 4, 512], F16, tag="wo", name=f"wo{i}")
        src = dio[f"Wout{i}"].rearrange("(a p) c -> p a c", p=128)
        nc.sync.dma_start(wo[i][:, :, :], src)

    def alloc_mod(i):
        kF[i] = [
            p_kq.tile([128, 2, 512], FP8, tag=f"kF{i}{g}", name=f"kF{i}{g}")
            for g in range(2)
        ]
        qF[i] = [
            p_kq.tile([128, 2, n_c], FP8, tag=f"qF{i}{g}", name=f"qF{i}{g}")
            for g in range(2)
        ]
        V[i] = p_v.tile([128, 4, 8, 65], F16, tag=f"V{i}", name=f"V{i}")
        nc.vector.memset(V[i][:, :, :, 64:65], 1.0)
        oT[i] = p_oT.tile([128, 4, n_c], F16, tag=f"oT{i}", name=f"oT{i}")

    def proj_ps(kind, name):
        # "A": prologue passes double-buffer through the idle psA pool;
        # "B": the psB "mm" slot; "C": the psB "avo" slot (free until the
        # first AV accumulation opens) -- B/C alternation double-buffers
        # the start-region passes.
        if kind == "A":
            t = psA.tile([128, 3, 512], F32, tag="dots", name=name)
            return t[:, 0, :]
        tag = "avo" if kind == "C" else "mm"
        return psB.tile([128, 512], F32, tag=tag, name=name)

    def k_pass(i, g, j, pk="B"):
        # fp8 DoubleRow: contraction over d pairs kt-chunks; Wk8 is sent
        # pre-scaled x32 so kF = 0.5*psum carries the same x16 as qF.
        # Split into column halves so the first psum->SBUF copy overlaps
        # the second half's matmuls (the "mm" slot is single-buffered).
        ps = proj_ps(pk, f"psk{i}{g}{j}")
        wsl = wk8[i][:, :, (2 * g + j) * 128 : (2 * g + j + 1) * 128]
        for half in range(2):
            sl = slice(half * 256, (half + 1) * 256)
            for kp in range(2):
                mm(ps[:, sl],
                   wsl[:, 2 * kp : 2 * kp + 2, :],
                   xt8[i][:, 2 * kp : 2 * kp + 2, sl],
                   start=kp == 0, stop=kp == 1, perf_mode=DR)
            nc.vector.tensor_scalar_mul(kF[i][g][:, j, sl], ps[:, sl], 0.5)

    def q_pass(i, g, j, pk="B", act_copy=False):
        # act_copy: route the odd half's psum->fp8 copy through ScalarE
        # (idle in the prologue) so DVE isn't the pass serializer.
        ps = proj_ps(pk, f"psq{i}{g}{j}")
        wsl = w[i][:, :, 512 + (2 * g + j) * 128 : 512 + (2 * g + j + 1) * 128]
        h1 = n_c // 2
        for half in range(2):
            sl = slice(half * h1, (half + 1) * h1 if half == 0 else n_c)
            for kt in range(4):
                mm(ps[:, sl], wsl[:, kt, :], xqt[i][:, kt, sl],
                   start=kt == 0, stop=kt == 3)
            if act_copy and half == 1:
                nc.scalar.mul(qF[i][g][:, j, sl], ps[:, sl], QK_SCALE)
            else:
                nc.vector.tensor_scalar_mul(qF[i][g][:, j, sl], ps[:, sl], QK_SCALE)

    def v_pass(i, mch, pk="B"):
        ps = proj_ps(pk, f"psv{i}{mch}")
        for half in range(2):
            sl = slice(half * 256, (half + 1) * 256)
            for kt in range(4):
                mm(ps[:, sl], xt[i][:, kt, mch * 128 : (mch + 1) * 128],
                   w[i][:, kt, 1024 + half * 256 : 1280 + half * 256],
                   start=kt == 0, stop=kt == 3)
            nc.vector.tensor_copy(
                V[i][:, mch, half * 4 : (half + 1) * 4, 0:64],
                ps[:, sl].rearrange("p (h x) -> p h x", x=64),
            )

    # Attention is split into a dots/exp part and a deferred AV part: the
    # dots of head h+1 are emitted before the AV of head h, so the in-order
    # PE keeps ScalarE fed while the AV accumulator slot ("avo", single
    # buffered) drains through DVE/Pool.
    chunks = [(j_, c) for j_ in range(3) for c in range(4)]
    pending_ex = {}
    pending_pso = {}

    def head_dots(i, h, between=None):
        g, hh = h // 4, h % 4
        kslc = lambda j_, c: kF[j_][g][hh * 32 : hh * 32 + 32, :, c * 128 : (c + 1) * 128]
        qslc = qF[i][g][hh * 32 : hh * 32 + 32, :, :]
        exs = []
        pending_ex[(i, h)] = exs  # filled in place; av parts may read early
        for t in range(4):
            ps_d = psA.tile([128, 3, 512], F32, tag="dots", name=f"psd{i}{h}{t}")
            for u in range(3):
                j_, c = chunks[t * 3 + u]
                mm(ps_d[:, u, 0:n_c], kslc(j_, c), qslc,
                   start=True, stop=True, perf_mode=DR,
                   tile_position=(hh * 32, 0))
            if between is not None and t < len(between) and between[t] is not None:
                for fn in between[t]:
                    fn()
            ex = p_ex.tile([128, 3, n_c], F16, tag="ex", name=f"ex{i}{h}{t}")
            nc.scalar.activation(
                ex[:, :, :], ps_d[:, :, 0:n_c], Exp, scale=SCALE / (QK_SCALE**2)
            )
            exs.append(ex)

    def av_part(i, h, j_):
        # one modality's worth (4 chunks) of the AV accumulation
        if j_ == 0:
            pending_pso[(i, h)] = psB.tile(
                [128, 512], F32, tag="avo", name=f"pso{i}{h}"
            )
        ps_o = pending_pso[(i, h)]
        exs = pending_ex[(i, h)]
        for c in range(4):
            t, u = divmod(j_ * 4 + c, 3)
            mm(ps_o[0:65, 0:n_c], V[j_][:, c, h, :], exs[t][:, u, :],
               start=j_ == 0 and c == 0, stop=j_ == 2 and c == 3)

    def av_drain(i, h):
        # copy psum -> SBUF f16 first: frees the single "avo" psum slot after
        # ~300ns instead of holding it through the recip/broadcast/mul chain
        ps_o = pending_pso.pop((i, h))
        pending_ex.pop((i, h))
        oc = p_oc.tile([65, 512], F16, tag="oc", name=f"oc{i}{h}")
        nc.vector.tensor_copy(oc[:, 0:n_c], ps_o[0:65, 0:n_c])
        den_s = p_den.tile([1, 512], F32, tag="dens", name=f"ds{i}{h}")
        nc.vector.reciprocal(den_s[0:1, 0:n_c], oc[64:65, 0:n_c])
        den_r = p_den.tile([64, 512], F32, tag="denr", name=f"dr{i}{h}")
        nc.gpsimd.partition_broadcast(den_r[:, 0:n_c], den_s[0:1, 0:n_c])
        po, fh = (h % 2) * 64, h // 2
        nc.vector.tensor_mul(
            oT[i][po : po + 64, fh, :], oc[0:64, 0:n_c], den_r[:, 0:n_c]
        )

    def head_av(i, h):
        for j_ in range(3):
            av_part(i, h, j_)
        av_drain(i, h)

    ob_t = {}

    def outproj_chunk(i, s, nn, pk="B"):
        # f16 output staged into one [128, 3, 512] tile; chunks 0+1 go out in
        # a single DMA (dram padded to 384 rows), the 19-row tail separately.
        if s == 0:
            ob_t[i] = p_ob.tile([128, 3, 512], F16, tag="ob", name=f"ob{i}")
        ps = proj_ps(pk, f"psf{i}{s}")
        ci = s // 128
        for half in range(2):
            sl = slice(half * 256, (half + 1) * 256)
            for ct in range(4):
                mm(ps[0:nn, sl], oT[i][:, ct, s : s + nn], wo[i][:, ct, sl],
                   start=ct == 0, stop=ct == 3)
            nc.vector.tensor_copy(ob_t[i][0:nn, ci, sl], ps[0:nn, sl])
        dst = dio[f"out{i}"].rearrange("(c p) d -> p c d", p=128)
        if ci == 1:
            nc.sync.dma_start(dst[:, 0:2, :], ob_t[i][:, 0:2, :])
        elif ci == 2:
            nc.sync.dma_start(dio[f"out{i}"][256 : 256 + nn, :], ob_t[i][0:nn, 2, :])

    # ---------------- schedule ----------------
    # DMA emission order = need order: K-g0 cols of all mods first (the
    # prologue), then Q0-g0/V0, V1/V2 (extras of head (0,0)), K-g1
    # (extras (0,1)-(0,3)), then the rest.
    for i in range(3):
        alloc_inputs(i)
    dma_wk8(0, 0, 256)   # K g0 operands (fp8 DR) for the prologue
    dma_xt8(0)
    dma_w(0, 512, 768)   # Q0 g0
    dma_xqt(0)
    dma_wk8(1, 0, 256)
    dma_xt8(1)
    dma_wk8(2, 0, 256)
    dma_xt8(2)
    dma_xt(0)            # V operands (f16 x + W V cols)
    dma_w(0, 1024, 1536)
    dma_xt(1)
    dma_w(1, 1024, 1536)
    dma_xt(2)
    dma_w(2, 1024, 1536)
    dma_w(1, 512, 1024)  # Q1 (needed at the (0,3)->(1,0) boundary)
    dma_xqt(1)
    for i in range(3):   # K g1
        dma_wk8(i, 256, 512)
    dma_w(0, 768, 1024)
    dma_w(2, 512, 1024)
    dma_xqt(2)
    load_wo(0)
    load_wo(1)

    # PE warm-up: dependency-free matmuls burn the p-state ramp while the
    # first DMAs land, so the projection passes run at full clock.  Emitted
    # before the V memsets so the warm tile's memset is DVE's first op and
    # the ramp starts ~1.2us in; 38 reps end right as the K operands land.
    warm = p_den.tile([128, 256], F16, tag="warm", name="warm")
    nc.vector.memset(warm[:, :], 0.0)
    ps_warm = psB.tile([128, 512], F32, tag="mm", name="ps_warm")
    for r in range(38):
        mm(ps_warm[:, 0:64], warm[:, 0:128], warm[:, 0:64],
           start=True, stop=True)

    for i in range(3):
        alloc_mod(i)

    # Start region: dots of heads (0,0)-(0,3) go out as early as possible;
    # K passes of mods 1/2 and all V passes are woven into the between-
    # group slots (PE work there does not delay ScalarE, which only needs
    # the dots).  The first AV is split per modality behind the V passes.
    for j in range(2):
        k_pass(0, 0, j, pk="A")
    for j in range(2):
        q_pass(0, 0, j, pk="A")
    head_dots(0, 0, between=[
        [lambda: k_pass(1, 0, 0, pk="B"), lambda: k_pass(1, 0, 1, pk="C")],
        [lambda: k_pass(2, 0, 0, pk="B"), lambda: k_pass(2, 0, 1, pk="C")],
        None, None,
    ])
    head_dots(0, 1, between=[
        [lambda: v_pass(0, 0, pk="B")], [lambda: v_pass(0, 1, pk="C")],
        [lambda: v_pass(0, 2, pk="B")], [lambda: v_pass(0, 3, pk="C")],
    ])
    head_dots(0, 2, between=[
        [lambda: v_pass(1, 0, pk="B")], [lambda: v_pass(1, 1, pk="C")],
        [lambda: v_pass(1, 2, pk="B")], [lambda: v_pass(1, 3, pk="C")],
    ])
    head_dots(0, 3, between=[
        [lambda: v_pass(2, 0, pk="B")], [lambda: v_pass(2, 1, pk="C")],
        [lambda: v_pass(2, 2, pk="B")], [lambda: v_pass(2, 3, pk="C")],
    ])
    q_pass(1, 0, 0, pk="B")  # Q1 g0: needed by dots(1,0), the next window
    q_pass(1, 0, 1, pk="C")
    av_part(0, 0, 0)
    av_part(0, 0, 1)
    av_part(0, 0, 2)
    av_drain(0, 0)

    # Remaining head order: all group-0 heads of the 3 mods, then all
    # group-1 heads -- spreads the g1 projection passes thinly over many
    # windows so ScalarE stays fed.  Lag-1 AV deferral via pending queue.
    seq = [
        (i, h)
        for g_ in range(2)
        for i in range(3)
        for h in range(4 * g_, 4 * g_ + 4)
    ][4:]  # (0,0)-(0,3) already emitted above

    # extra PE work per window; every dependency lands >= 1 window early:
    #   Q1g0 before dots(1,0);  Q2g0 before dots(2,0);
    #   K g1 (all mods) + Q0g1 before dots(0,4);  Q1g1 before (1,4);
    #   Q2g1 before (2,4);  outproj(i) after drain(i,7) [lag-1].
    nch_list = [(s, min(128, n_c - s)) for s in range(0, n_c, 128)]
    extras = {
        (1, 0): [lambda j=j: q_pass(2, 0, j) for j in range(2)],
        (1, 2): [lambda j=j: k_pass(0, 1, j) for j in range(2)],
        (1, 3): [lambda j=j: k_pass(1, 1, j) for j in range(2)],
        (2, 0): [lambda j=j: k_pass(2, 1, j) for j in range(2)],
        (2, 1): [lambda j=j: q_pass(0, 1, j) for j in range(2)],
        (2, 2): [lambda j=j: q_pass(1, 1, j) for j in range(2)],
        (2, 3): [lambda j=j: q_pass(2, 1, j) for j in range(2)],
        (2, 4): [lambda: load_wo(2)],
    }
    for k_, (s, nn) in enumerate(nch_list):
        extras[(1, 5 + k_)] = [lambda i=0, s=s, nn=nn: outproj_chunk(i, s, nn)]
        extras[(2, 5 + k_)] = [lambda i=1, s=s, nn=nn: outproj_chunk(i, s, nn)]

    pending = [(0, 1), (0, 2), (0, 3)]
    for i, h in seq[:-1]:
        head_dots(i, h)
        for fn in extras.get((i, h), []):
            fn()
        pending.append((i, h))
        head_av(*pending.pop(0))
        if (i, h) in ((1, 1), (1, 2)) and len(pending) > 1:
            head_av(*pending.pop(0))  # catch up to lag-1 in light windows
    # Last head: the previous head's AV and its own AV parts are woven into
    # its dots groups so the post-exp tail is just the final 4 AV matmuls +
    # drain + out-proj.
    head_dots(2, 7, between=[
        [lambda: head_av(*pending.pop(0))],
        [lambda: head_av(*pending.pop(0))] if len(pending) > 1 else None,
        [lambda: av_part(2, 7, 0)],
        [lambda: av_part(2, 7, 1)],
    ])
    for fn in extras.get((2, 7), []):
        fn()
    av_part(2, 7, 2)
    av_drain(2, 7)
    for s, nn in nch_list:
        outproj_chunk(2, s, nn, pk="A")


def build(n_c=275, reps=1):
    assert 256 < n_c <= 384, "out-DMA chunking hardcodes the 3-chunk case"
    nc = bacc.Bacc("TRN2", target_bir_lowering=False, debug=False)
    dio = {}
    for i in range(3):
        dio[f"xT{i}"] = nc.dram_tensor(f"xT{i}", [D, N], F16, kind="ExternalInput").ap()
        dio[f"xT8{i}"] = nc.dram_tensor(
            f"xT8{i}", [D, N], FP8, kind="ExternalInput"
        ).ap()
        dio[f"Wk8{i}"] = nc.dram_tensor(
            f"Wk8{i}", [D, DI], FP8, kind="ExternalInput"
        ).ap()
        dio[f"xqT{i}"] = nc.dram_tensor(
            f"xqT{i}", [D, n_c], F16, kind="ExternalInput"
        ).ap()
        dio[f"W{i}"] = nc.dram_tensor(
            f"W{i}", [D, 3 * DI], F16, kind="ExternalInput"
        ).ap()
        dio[f"Wout{i}"] = nc.dram_tensor(
            f"Wout{i}", [DI, D], F16, kind="ExternalInput"
        ).ap()
        dio[f"out{i}"] = nc.dram_tensor(
            f"out{i}", [384, D], F16, kind="ExternalOutput"
        ).ap()
    with tile.TileContext(nc) as tc:
        with (
            tc.tile_pool(name="w", bufs=1) as p_w,
            tc.tile_pool(name="x", bufs=1) as p_x,
            tc.tile_pool(name="xq", bufs=1) as p_xq,
            tc.tile_pool(name="kq", bufs=1) as p_kq,
            tc.tile_pool(name="v", bufs=1) as p_v,
            tc.tile_pool(name="ex", bufs=24) as p_ex,
            tc.tile_pool(name="oT", bufs=1) as p_oT,
            tc.tile_pool(name="wo", bufs=3) as p_wo,
            tc.tile_pool(name="ob", bufs=2) as p_ob,
            tc.tile_pool(name="oc", bufs=3) as p_oc,
            tc.tile_pool(name="den", bufs=4) as p_den,
            tc.tile_pool(name="psA", bufs=2, space="PSUM") as psA,
            tc.tile_pool(name="psB", bufs=1, space="PSUM") as psB,
        ):
            pools = (
                p_w, p_x, p_xq, p_kq, p_v, p_ex, p_oT, p_wo, p_ob, p_oc,
                p_den, psA, psB,
            )
            for _ in range(reps):
                _emit_body(nc, tc, dio, pools, n_c)
    nc.compile()
    return nc


_BUILD_CACHE = {}


def _get_built(n_c, reps):
    key = (n_c, reps)
    if key not in _BUILD_CACHE:
        _BUILD_CACHE[key] = build(n_c, reps)
    return _BUILD_CACHE[key]


def pick_n_c(inputs):
    """Smallest compacted-query count: unmasked rows + 1 dummy zero row."""
    need = 0
    for i in range(3):
        m = np.asarray(inputs[f"m{i}"]).astype(bool)
        for b in range(B):
            n_u = int(m[b].sum())
            need = max(need, n_u + (1 if n_u < N else 0))
    return max(need, 192)


WK8_SCALE = 32.0  # Wk pre-scale into fp8 range; kF copy multiplies by
                  # QK_SCALE/WK8_SCALE so kF carries the same x16 as qF


def make_in_maps(inputs, n_c=275):
    """Per-core input dicts: transposed fp16 x (+fp8 copy for the K
    DoubleRow projection), compacted fp16 xq, column-permuted fp16 Wqkv,
    fp8 x32 K-weights, fp16 Wout."""
    xs = [np.asarray(inputs[f"x{i}"], dtype=np.float32) for i in range(3)]
    ms = [np.asarray(inputs[f"m{i}"]).astype(bool) for i in range(3)]
    Wp = [
        np.asarray(inputs[f"Wqkv{i}"], np.float32)[:, W_PERM].astype(NP_F16)
        for i in range(3)
    ]
    Wk8 = [
        np.ascontiguousarray(
            WK8_SCALE
            * np.asarray(inputs[f"Wqkv{i}"], np.float32)[:, W_PERM[0:DI]]
        ).astype(NP_FP8)
        for i in range(3)
    ]
    Wo = [np.asarray(inputs[f"Wout{i}"], np.float32).astype(NP_F16) for i in range(3)]
    in_maps = []
    for b in range(B):
        m = {}
        for i in range(3):
            xb = xs[i][b]
            xbT = np.ascontiguousarray(xb.T)
            m[f"xT{i}"] = xbT.astype(NP_F16)
            m[f"xT8{i}"] = xbT.astype(NP_FP8)
            sel = np.flatnonzero(ms[i][b])
            xq = np.zeros((n_c, D), np.float32)
            xq[: len(sel)] = xb[sel[:n_c]]
            m[f"xqT{i}"] = np.ascontiguousarray(xq.T.astype(NP_F16))
            m[f"W{i}"] = Wp[i]
            m[f"Wk8{i}"] = Wk8[i]
            m[f"Wout{i}"] = Wo[i]
        in_maps.append(m)
    return in_maps


def scatter_outputs(results, inputs, n_c):
    ms = [np.asarray(inputs[f"m{i}"]).astype(bool) for i in range(3)]
    outs = []
    for i in range(3):
        full = np.empty((B, N, D), np.float32)
        for b in range(B):
            comp = np.asarray(results[b][f"out{i}"], np.float32)
            sel = np.flatnonzero(ms[i][b])
            full[b][sel] = comp[: len(sel)]
            if len(sel) < N:
                full[b][~ms[i][b]] = comp[len(sel)]
        outs.append(full)
    return outs


def kernel(**inputs):
    n_c = pick_n_c(inputs)
    in_maps = make_in_maps(inputs, n_c)
    nc = _get_built(n_c=n_c, reps=1)
    res = bass_utils.run_bass_kernel_spmd(nc, in_maps, core_ids=list(range(N_CORES)))
    return tuple(scatter_outputs(res.results, inputs, n_c))



# revision 12
# speedup vs baseline: 1.1034x; 1.0338x over previous
"""Trainium2 Bass kernel for nn_JointModalityAttention.

3-modality joint attention, B=8, N=512, D=512, 8 heads x 64.
Sharding: data-parallel over batch -- each of the 8 NeuronCores handles one
batch element; weights replicated; no collectives.

v2 design (vs v1 f32r baseline):
  - Query compaction to n_c=275 (unmasked rows + 1 dummy zero row whose
    uniform-attention output serves every masked row).
  - All projections + AV + out-projection in fp16 (same PE rate as f32r,
    half the DMA bytes).
  - dots in fp8e4 DoubleRow: qT/kT are built in a "folded" layout
    [32 partitions, 2, n] (dh = p + 32j) so one DR matmul contracts the
    full 64-dim head axis at 0.5 cycles/row.  The fold comes free by
    permuting W's columns host-side; q/k are scaled x16 into fp8's sweet
    range and the 1/256 is folded into the exp scale.
  - exp on ScalarE reads the whole 3-chunk PSUM group in one instruction,
    writes fp16 ex tiles; denominator via the ones-column of V_ext
    (row 64 of the AV accumulation).
  - PE program order interleaves mod i+1 projections / mod i-1 out-
    projections into attention heads so the PE fills ScalarE-bound gaps.
"""

import sys

if "/opt/trn_rl_repo" not in sys.path:
    sys.path.insert(0, "/opt/trn_rl_repo")

import ml_dtypes
import numpy as np

import concourse.bass as bass  # noqa: F401  (import keeps bass registered)
import concourse.mybir as mybir
import concourse.tile as tile
from concourse import bacc, bass_utils

HEADS = 8
DH = 64
DI = HEADS * DH  # 512
B = 8
N = 512
D = 512
SCALE = DH ** -0.5
N_CORES = 8
QK_SCALE = 16.0  # q/k pre-scale into fp8 range; folded out in exp scale

F32 = mybir.dt.float32
F16 = mybir.dt.float16
FP8 = mybir.dt.float8e4
DR = mybir.MatmulPerfMode.DoubleRow
NP_F16 = np.float16
NP_FP8 = ml_dtypes.float8_e4m3


def qk_pass_channels(base):
    """Channel order for the 4 folded QK projection passes (g, j).

    Pass (g, j) covers heads 4g..4g+3; PSUM partition p holds channel
    base + head(g,p)*64 + (p%32) + 32*j, so the SBUF tile [128, 2, n]
    (partition p, free j) is exactly the DR lhsT/rhs fold (dh = p%32+32j,
    head = p//32 within group).
    """
    cols = []
    for g in range(2):
        for j in range(2):
            for p in range(128):
                head = 4 * g + p // 32
                dh = (p % 32) + 32 * j
                cols.append(base + head * 64 + dh)
    return cols


def w_perm_cols():
    """Column permutation for the device Wqkv tensor: 4 K passes, 4 Q
    passes (128 cols each), then V natural."""
    return qk_pass_channels(DI) + qk_pass_channels(0) + list(range(2 * DI, 3 * DI))


W_PERM = np.array(w_perm_cols())


def _emit_body(nc, tc, dio, pools, n_c):
    (p_w, p_x, p_xq, p_kq, p_v, p_ex, p_oT, p_wo, p_ob, p_oc,
     p_den, psA, psB) = pools
    Exp = mybir.ActivationFunctionType.Exp
    mm = nc.tensor.matmul

    w, xt, xqt, wo = {}, {}, {}, {}
    kF, qF, V, oT = {}, {}, {}, {}

    xt8, wk8 = {}, {}

    def alloc_inputs(i):
        w[i] = p_w.tile([128, 4, 1536], F16, tag=f"w{i}", name=f"w{i}")
        xt[i] = p_x.tile([128, 4, 512], F16, tag=f"xt{i}", name=f"xt{i}")
        xt8[i] = p_x.tile([128, 4, 512], FP8, tag=f"xt8{i}", name=f"xt8{i}")
        wk8[i] = p_x.tile([128, 4, 512], FP8, tag=f"wk8{i}", name=f"wk8{i}")
        xqt[i] = p_xq.tile([128, 4, n_c], F16, tag=f"xqt{i}", name=f"xqt{i}")

    # one DMA instruction per logical block: each instruction costs a fixed
    # ~625 ns on the serial HWDGE device, so instruction count is what matters
    def dma_w(i, c0, c1):
        src = dio[f"W{i}"].rearrange("(a p) c -> p a c", p=128)
        nc.sync.dma_start(w[i][:, :, c0:c1], src[:, :, c0:c1])

    def dma_xt(i):
        src = dio[f"xT{i}"].rearrange("(a p) c -> p a c", p=128)
        nc.sync.dma_start(xt[i][:, :, :], src)

    def dma_xt8(i):
        src8 = dio[f"xT8{i}"].rearrange("(a p) c -> p a c", p=128)
        nc.sync.dma_start(xt8[i][:, :, :], src8)

    def dma_wk8(i, c0, c1):
        srcw = dio[f"Wk8{i}"].rearrange("(a p) c -> p a c", p=128)
        nc.sync.dma_start(wk8[i][:, :, c0:c1], srcw[:, :, c0:c1])

    def dma_xqt(i):
        src = dio[f"xqT{i}"].rearrange("(a p) c -> p a c", p=128)
        nc.sync.dma_start(xqt[i][:, :, :], src)

    def load_wo(i):
        wo[i] = p_wo.tile([128, 4, 512], F16, tag="wo", name=f"wo{i}")
        src = dio[f"Wout{i}"].rearrange("(a p) c -> p a c", p=128)
        nc.sync.dma_start(wo[i][:, :, :], src)

    def alloc_mod(i):
        kF[i] = [
            p_kq.tile([128, 2, 512], FP8, tag=f"kF{i}{g}", name=f"kF{i}{g}")
            for g in range(2)
        ]
        qF[i] = [
            p_kq.tile([128, 2, n_c], FP8, tag=f"qF{i}{g}", name=f"qF{i}{g}")
            for g in range(2)
        ]
        V[i] = p_v.tile([128, 4, 8, 65], F16, tag=f"V{i}", name=f"V{i}")
        nc.vector.memset(V[i][:, :, :, 64:65], 1.0)
        oT[i] = p_oT.tile([128, 4, n_c], F16, tag=f"oT{i}", name=f"oT{i}")

    def proj_ps(kind, name):
        # "A": prologue passes double-buffer through the idle psA pool;
        # "B": the psB "mm" slot; "C": the psB "avo" slot (free until the
        # first AV accumulation opens) -- B/C alternation double-buffers
        # the start-region passes.
        if kind == "A":
            t = psA.tile([128, 3, 512], F32, tag="dots", name=name)
            return t[:, 0, :]
        tag = "avo" if kind == "C" else "mm"
        return psB.tile([128, 512], F32, tag=tag, name=name)

    def k_pass(i, g, j, pk="B"):
        # fp8 DoubleRow: contraction over d pairs kt-chunks; Wk8 is sent
        # pre-scaled x32 so kF = 0.5*psum carries the same x16 as qF.
        # Split into column halves so the first psum->SBUF copy overlaps
        # the second half's matmuls (the "mm" slot is single-buffered).
        ps = proj_ps(pk, f"psk{i}{g}{j}")
        wsl = wk8[i][:, :, (2 * g + j) * 128 : (2 * g + j + 1) * 128]
        for half in range(2):
            sl = slice(half * 256, (half + 1) * 256)
            for kp in range(2):
                mm(ps[:, sl],
                   wsl[:, 2 * kp : 2 * kp + 2, :],
                   xt8[i][:, 2 * kp : 2 * kp + 2, sl],
                   start=kp == 0, stop=kp == 1, perf_mode=DR)
            nc.vector.tensor_scalar_mul(kF[i][g][:, j, sl], ps[:, sl], 0.5)

    def q_pass(i, g, j, pk="B", act_copy=False):
        # act_copy: route the odd half's psum->fp8 copy through ScalarE
        # (idle in the prologue) so DVE isn't the pass serializer.
        ps = proj_ps(pk, f"psq{i}{g}{j}")
        wsl = w[i][:, :, 512 + (2 * g + j) * 128 : 512 + (2 * g + j + 1) * 128]
        h1 = n_c // 2
        for half in range(2):
            sl = slice(half * h1, (half + 1) * h1 if half == 0 else n_c)
            for kt in range(4):
                mm(ps[:, sl], wsl[:, kt, :], xqt[i][:, kt, sl],
                   start=kt == 0, stop=kt == 3)
            if act_copy and half == 1:
                nc.scalar.mul(qF[i][g][:, j, sl], ps[:, sl], QK_SCALE)
            else:
                nc.vector.tensor_scalar_mul(qF[i][g][:, j, sl], ps[:, sl], QK_SCALE)

    def v_pass(i, mch, pk="B"):
        ps = proj_ps(pk, f"psv{i}{mch}")
        for half in range(2):
            sl = slice(half * 256, (half + 1) * 256)
            for kt in range(4):
                mm(ps[:, sl], xt[i][:, kt, mch * 128 : (mch + 1) * 128],
                   w[i][:, kt, 1024 + half * 256 : 1280 + half * 256],
                   start=kt == 0, stop=kt == 3)
            nc.vector.tensor_copy(
                V[i][:, mch, half * 4 : (half + 1) * 4, 0:64],
                ps[:, sl].rearrange("p (h x) -> p h x", x=64),
            )

    # Attention is split into a dots/exp part and a deferred AV part: the
    # dots of head h+1 are emitted before the AV of head h, so the in-order
    # PE keeps ScalarE fed while the AV accumulator slot ("avo", single
    # buffered) drains through DVE/Pool.
    chunks = [(j_, c) for j_ in range(3) for c in range(4)]
    pending_ex = {}
    pending_pso = {}

    def head_dots(i, h, between=None):
        g, hh = h // 4, h % 4
        kslc = lambda j_, c: kF[j_][g][hh * 32 : hh * 32 + 32, :, c * 128 : (c + 1) * 128]
        qslc = qF[i][g][hh * 32 : hh * 32 + 32, :, :]
        exs = []
        pending_ex[(i, h)] = exs  # filled in place; av parts may read early
        for t in range(4):
            ps_d = psA.tile([128, 3, 512], F32, tag="dots", name=f"psd{i}{h}{t}")
            for u in range(3):
                j_, c = chunks[t * 3 + u]
                mm(ps_d[:, u, 0:n_c], kslc(j_, c), qslc,
                   start=True, stop=True, perf_mode=DR,
                   tile_position=(hh * 32, 0))
            if between is not None and t < len(between) and between[t] is not None:
                for fn in between[t]:
                    fn()
            ex = p_ex.tile([128, 3, n_c], F16, tag="ex", name=f"ex{i}{h}{t}")
            nc.scalar.activation(
                ex[:, :, :], ps_d[:, :, 0:n_c], Exp, scale=SCALE / (QK_SCALE**2)
            )
            exs.append(ex)

    def av_part(i, h, j_):
        # one modality's worth (4 chunks) of the AV accumulation
        if j_ == 0:
            pending_pso[(i, h)] = psB.tile(
                [128, 512], F32, tag="avo", name=f"pso{i}{h}"
            )
        ps_o = pending_pso[(i, h)]
        exs = pending_ex[(i, h)]
        for c in range(4):
            t, u = divmod(j_ * 4 + c, 3)
            mm(ps_o[0:65, 0:n_c], V[j_][:, c, h, :], exs[t][:, u, :],
               start=j_ == 0 and c == 0, stop=j_ == 2 and c == 3)

    def av_drain(i, h):
        # copy psum -> SBUF f16 first: frees the single "avo" psum slot after
        # ~300ns instead of holding it through the recip/broadcast/mul chain
        ps_o = pending_pso.pop((i, h))
        pending_ex.pop((i, h))
        oc = p_oc.tile([65, 512], F16, tag="oc", name=f"oc{i}{h}")
        nc.vector.tensor_copy(oc[:, 0:n_c], ps_o[0:65, 0:n_c])
        den_s = p_den.tile([1, 512], F32, tag="dens", name=f"ds{i}{h}")
        nc.vector.reciprocal(den_s[0:1, 0:n_c], oc[64:65, 0:n_c])
        den_r = p_den.tile([64, 512], F32, tag="denr", name=f"dr{i}{h}")
        nc.gpsimd.partition_broadcast(den_r[:, 0:n_c], den_s[0:1, 0:n_c])
        po, fh = (h % 2) * 64, h // 2
        nc.vector.tensor_mul(
            oT[i][po : po + 64, fh, :], oc[0:64, 0:n_c], den_r[:, 0:n_c]
        )

    def head_av(i, h):
        for j_ in range(3):
            av_part(i, h, j_)
        av_drain(i, h)

    ob_t = {}

    def outproj_chunk(i, s, nn, pk="B"):
        # f16 output staged into one [128, 3, 512] tile; chunks 0+1 go out in
        # a single DMA (dram padded to 384 rows), the 19-row tail separately.
        if s == 0:
            ob_t[i] = p_ob.tile([128, 3, 512], F16, tag="ob", name=f"ob{i}")
        ps = proj_ps(pk, f"psf{i}{s}")
        ci = s // 128
        for half in range(2):
            sl = slice(half * 256, (half + 1) * 256)
            for ct in range(4):
                mm(ps[0:nn, sl], oT[i][:, ct, s : s + nn], wo[i][:, ct, sl],
                   start=ct == 0, stop=ct == 3)
            nc.vector.tensor_copy(ob_t[i][0:nn, ci, sl], ps[0:nn, sl])
        dst = dio[f"out{i}"].rearrange("(c p) d -> p c d", p=128)
        if ci == 1:
            nc.sync.dma_start(dst[:, 0:2, :], ob_t[i][:, 0:2, :])
        elif ci == 2:
            nc.sync.dma_start(dio[f"out{i}"][256 : 256 + nn, :], ob_t[i][0:nn, 2, :])

    # ---------------- schedule ----------------
    # DMA emission order = need order: K-g0 cols of all mods first (the
    # prologue), then Q0-g0/V0, V1/V2 (extras of head (0,0)), K-g1
    # (extras (0,1)-(0,3)), then the rest.
    for i in range(3):
        alloc_inputs(i)
    dma_wk8(0, 0, 256)   # K g0 operands (fp8 DR) for the prologue
    dma_xt8(0)
    dma_w(0, 512, 768)   # Q0 g0
    dma_xqt(0)
    dma_wk8(1, 0, 256)
    dma_xt8(1)
    dma_wk8(2, 0, 256)
    dma_xt8(2)
    dma_xt(0)            # V operands (f16 x + W V cols)
    dma_w(0, 1024, 1536)
    dma_xt(1)
    dma_w(1, 1024, 1536)
    dma_xt(2)
    dma_w(2, 1024, 1536)
    dma_w(1, 512, 1024)  # Q1 (needed at the (0,3)->(1,0) boundary)
    dma_xqt(1)
    for i in range(3):   # K g1
        dma_wk8(i, 256, 512)
    dma_w(0, 768, 1024)
    dma_w(2, 512, 1024)
    dma_xqt(2)
    load_wo(0)
    load_wo(1)

    # PE warm-up: dependency-free matmuls burn the p-state ramp while the
    # first DMAs land, so the projection passes run at full clock.  Emitted
    # before the V memsets so the warm tile's memset is DVE's first op and
    # the ramp starts ~1.2us in; 38 reps end right as the K operands land.
    warm = p_den.tile([128, 256], F16, tag="warm", name="warm")
    nc.vector.memset(warm[:, :], 0.0)
    ps_warm = psB.tile([128, 512], F32, tag="mm", name="ps_warm")
    for r in range(38):
        mm(ps_warm[:, 0:64], warm[:, 0:128], warm[:, 0:64],
           start=True, stop=True)

    for i in range(3):
        alloc_mod(i)

    # ------------- slot-pipelined main schedule -------------
    # The exp chain is the backbone: each window (i, h) emits its 4 dots
    # groups with all other PE work (pending heads' AV + projection passes +
    # out-projections) split across the 4 between-slots, so the next dots
    # group is always near the head of PE's in-order queue when its psA
    # buffer frees.  Filler is hand-placed per window; ~2.8us capacity each
    # (4 exp slots of 873ns minus 684ns of dots).
    for j in range(2):
        k_pass(0, 0, j, pk="A")
    for j in range(2):
        q_pass(0, 0, j, pk="A")

    nch_list = [(s, min(128, n_c - s)) for s in range(0, n_c, 128)]
    AB = ("B", "C")  # pre-AV psB double-buffer alternation
    BB = ("B", "B")  # post-AV: "C" (avo) is owned by AV accumulation

    def kp(i, g, pks=BB):
        return [(212, lambda j=j, p=pks[j]: k_pass(i, g, j, pk=p))
                for j in range(2)]

    def qp(i, g, pks=BB):
        return [(458, lambda j=j, p=pks[j]: q_pass(i, g, j, pk=p))
                for j in range(2)]

    def vp(i, c, pk):
        return [(856, lambda: v_pass(i, c, pk=pk))]

    def av4(i, h):
        its = [(460, lambda j_=j_: av_part(i, h, j_)) for j_ in range(3)]
        its.append((100, lambda: av_drain(i, h)))
        return its

    def op(i, k):
        s, nn = nch_list[k]
        return [(856, lambda: outproj_chunk(i, s, nn))]

    def zipav(av, other):
        out, a, b = [], list(av), list(other)
        while a or b:
            if b:
                out.append(b.pop(0))
            if a:
                out.append(a.pop(0))
        return out

    def split4(items):
        slots = [[], [], [], []]
        total = sum(c for c, _ in items)
        cum = 0
        for c, fn in items:
            slots[min(3, 4 * cum // max(total, 1))].append(fn)
            cum += c
        return slots

    # AV of head X drains in a later window (lag 1-5; ex pool holds 6 heads).
    # Projection placement: every dependency lands >= 1 window early; K g1 of
    # all mods done by (2,2) < first g1 window (0,4); Q(i)g1 done 4+ windows
    # before (i,4); outproj(i) right after drain(i,7).
    W = {
        (0, 0): kp(1, 0, AB) + kp(2, 0, AB),
        (0, 1): vp(0, 0, "B") + vp(0, 1, "C") + vp(0, 2, "B"),
        (0, 2): vp(0, 3, "C") + vp(1, 0, "B") + vp(1, 1, "C"),
        (0, 3): vp(1, 2, "B") + vp(1, 3, "C") + qp(1, 0, AB),
        (1, 0): vp(2, 0, "B") + vp(2, 1, "C") + vp(2, 2, "B"),
        (1, 1): vp(2, 3, "C") + av4(0, 0),
        (1, 2): zipav(av4(0, 1), qp(2, 0)),
        (1, 3): av4(0, 2) + av4(0, 3),
        (2, 0): zipav(av4(1, 0), kp(0, 1)),
        (2, 1): zipav(av4(1, 1), kp(1, 1)),
        (2, 2): zipav(av4(1, 2), kp(2, 1)),
        (2, 3): zipav(av4(1, 3), qp(0, 1)),
        (0, 4): zipav(av4(2, 0), qp(1, 1)),
        (0, 5): zipav(av4(2, 1), qp(2, 1)),
        (0, 6): av4(2, 2) + av4(2, 3),
        (0, 7): av4(0, 4) + [(50, lambda: load_wo(2))],
        (1, 4): av4(0, 5) + av4(0, 6),
        (1, 5): av4(0, 7) + op(0, 0),
        (1, 6): op(0, 1) + op(0, 2),
        (1, 7): av4(1, 4) + av4(1, 5),
        (2, 4): av4(1, 6) + av4(1, 7),
        (2, 5): zipav(av4(2, 4), op(1, 0) + op(1, 1)),
        (2, 6): zipav(av4(2, 5), op(1, 2)),
    }
    win_seq = [
        (i, h)
        for g_ in range(2)
        for i in range(3)
        for h in range(4 * g_, 4 * g_ + 4)
    ]
    for win in win_seq[:-1]:
        head_dots(*win, between=split4(W[win]))

    # Last window: AV of (2,6) in the early slots, (2,7)'s own AV parts as
    # soon as their exp groups are emitted, so the post-exp tail is just the
    # final 4 AV matmuls + drain + out-proj of mod 2.
    head_dots(2, 7, between=[
        [fn for _, fn in av4(2, 6)[:2]],
        [fn for _, fn in av4(2, 6)[2:]],
        [lambda: av_part(2, 7, 0)],
        [lambda: av_part(2, 7, 1)],
    ])
    av_part(2, 7, 2)
    av_drain(2, 7)
    for s, nn in nch_list:
        outproj_chunk(2, s, nn, pk="A")


def build(n_c=275, reps=1):
    assert 256 < n_c <= 384, "out-DMA chunking hardcodes the 3-chunk case"
    nc = bacc.Bacc("TRN2", target_bir_lowering=False, debug=False)
    dio = {}
    for i in range(3):
        dio[f"xT{i}"] = nc.dram_tensor(f"xT{i}", [D, N], F16, kind="ExternalInput").ap()
        dio[f"xT8{i}"] = nc.dram_tensor(
            f"xT8{i}", [D, N], FP8, kind="ExternalInput"
        ).ap()
        dio[f"Wk8{i}"] = nc.dram_tensor(
            f"Wk8{i}", [D, DI], FP8, kind="ExternalInput"
        ).ap()
        dio[f"xqT{i}"] = nc.dram_tensor(
            f"xqT{i}", [D, n_c], F16, kind="ExternalInput"
        ).ap()
        dio[f"W{i}"] = nc.dram_tensor(
            f"W{i}", [D, 3 * DI], F16, kind="ExternalInput"
        ).ap()
        dio[f"Wout{i}"] = nc.dram_tensor(
            f"Wout{i}", [DI, D], F16, kind="ExternalInput"
        ).ap()
        dio[f"out{i}"] = nc.dram_tensor(
            f"out{i}", [384, D], F16, kind="ExternalOutput"
        ).ap()
    with tile.TileContext(nc) as tc:
        with (
            tc.tile_pool(name="w", bufs=1) as p_w,
            tc.tile_pool(name="x", bufs=1) as p_x,
            tc.tile_pool(name="xq", bufs=1) as p_xq,
            tc.tile_pool(name="kq", bufs=1) as p_kq,
            tc.tile_pool(name="v", bufs=1) as p_v,
            tc.tile_pool(name="ex", bufs=24) as p_ex,
            tc.tile_pool(name="oT", bufs=1) as p_oT,
            tc.tile_pool(name="wo", bufs=3) as p_wo,
            tc.tile_pool(name="ob", bufs=2) as p_ob,
            tc.tile_pool(name="oc", bufs=3) as p_oc,
            tc.tile_pool(name="den", bufs=4) as p_den,
            tc.tile_pool(name="psA", bufs=2, space="PSUM") as psA,
            tc.tile_pool(name="psB", bufs=1, space="PSUM") as psB,
        ):
            pools = (
                p_w, p_x, p_xq, p_kq, p_v, p_ex, p_oT, p_wo, p_ob, p_oc,
                p_den, psA, psB,
            )
            for _ in range(reps):
                _emit_body(nc, tc, dio, pools, n_c)
    nc.compile()
    return nc


_BUILD_CACHE = {}


def _get_built(n_c, reps):
    key = (n_c, reps)
    if key not in _BUILD_CACHE:
        _BUILD_CACHE[key] = build(n_c, reps)
    return _BUILD_CACHE[key]


def pick_n_c(inputs):
    """Smallest compacted-query count: unmasked rows + 1 dummy zero row."""
    need = 0
    for i in range(3):
        m = np.asarray(inputs[f"m{i}"]).astype(bool)
        for b in range(B):
            n_u = int(m[b].sum())
            need = max(need, n_u + (1 if n_u < N else 0))
    return max(need, 192)


WK8_SCALE = 32.0  # Wk pre-scale into fp8 range; kF copy multiplies by
                  # QK_SCALE/WK8_SCALE so kF carries the same x16 as qF


def make_in_maps(inputs, n_c=275):
    """Per-core input dicts: transposed fp16 x (+fp8 copy for the K
    DoubleRow projection), compacted fp16 xq, column-permuted fp16 Wqkv,
    fp8 x32 K-weights, fp16 Wout."""
    xs = [np.asarray(inputs[f"x{i}"], dtype=np.float32) for i in range(3)]
    ms = [np.asarray(inputs[f"m{i}"]).astype(bool) for i in range(3)]
    Wp = [
        np.asarray(inputs[f"Wqkv{i}"], np.float32)[:, W_PERM].astype(NP_F16)
        for i in range(3)
    ]
    Wk8 = [
        np.ascontiguousarray(
            WK8_SCALE
            * np.asarray(inputs[f"Wqkv{i}"], np.float32)[:, W_PERM[0:DI]]
        ).astype(NP_FP8)
        for i in range(3)
    ]
    Wo = [np.asarray(inputs[f"Wout{i}"], np.float32).astype(NP_F16) for i in range(3)]
    in_maps = []
    for b in range(B):
        m = {}
        for i in range(3):
            xb = xs[i][b]
            xbT = np.ascontiguousarray(xb.T)
            m[f"xT{i}"] = xbT.astype(NP_F16)
            m[f"xT8{i}"] = xbT.astype(NP_FP8)
            sel = np.flatnonzero(ms[i][b])
            xq = np.zeros((n_c, D), np.float32)
            xq[: len(sel)] = xb[sel[:n_c]]
            m[f"xqT{i}"] = np.ascontiguousarray(xq.T.astype(NP_F16))
            m[f"W{i}"] = Wp[i]
            m[f"Wk8{i}"] = Wk8[i]
            m[f"Wout{i}"] = Wo[i]
        in_maps.append(m)
    return in_maps


def scatter_outputs(results, inputs, n_c):
    ms = [np.asarray(inputs[f"m{i}"]).astype(bool) for i in range(3)]
    outs = []
    for i in range(3):
        full = np.empty((B, N, D), np.float32)
        for b in range(B):
            comp = np.asarray(results[b][f"out{i}"], np.float32)
            sel = np.flatnonzero(ms[i][b])
            full[b][sel] = comp[: len(sel)]
            if len(sel) < N:
                full[b][~ms[i][b]] = comp[len(sel)]
        outs.append(full)
    return outs


def kernel(**inputs):
    n_c = pick_n_c(inputs)
    in_maps = make_in_maps(inputs, n_c)
    nc = _get_built(n_c=n_c, reps=1)
    res = bass_utils.run_bass_kernel_spmd(nc, in_maps, core_ids=list(range(N_CORES)))
    return tuple(scatter_outputs(res.results, inputs, n_c))



# revision 13
# speedup vs baseline: 1.1095x; 1.0055x over previous
"""Trainium2 Bass kernel for nn_JointModalityAttention.

3-modality joint attention, B=8, N=512, D=512, 8 heads x 64.
Sharding: data-parallel over batch -- each of the 8 NeuronCores handles one
batch element; weights replicated; no collectives.

v2 design (vs v1 f32r baseline):
  - Query compaction to n_c=275 (unmasked rows + 1 dummy zero row whose
    uniform-attention output serves every masked row).
  - All projections + AV + out-projection in fp16 (same PE rate as f32r,
    half the DMA bytes).
  - dots in fp8e4 DoubleRow: qT/kT are built in a "folded" layout
    [32 partitions, 2, n] (dh = p + 32j) so one DR matmul contracts the
    full 64-dim head axis at 0.5 cycles/row.  The fold comes free by
    permuting W's columns host-side; q/k are scaled x16 into fp8's sweet
    range and the 1/256 is folded into the exp scale.
  - exp on ScalarE reads the whole 3-chunk PSUM group in one instruction,
    writes fp16 ex tiles; denominator via the ones-column of V_ext
    (row 64 of the AV accumulation).
  - PE program order interleaves mod i+1 projections / mod i-1 out-
    projections into attention heads so the PE fills ScalarE-bound gaps.
"""

import sys

if "/opt/trn_rl_repo" not in sys.path:
    sys.path.insert(0, "/opt/trn_rl_repo")

import ml_dtypes
import numpy as np

import concourse.bass as bass  # noqa: F401  (import keeps bass registered)
import concourse.mybir as mybir
import concourse.tile as tile
from concourse import bacc, bass_utils

HEADS = 8
DH = 64
DI = HEADS * DH  # 512
B = 8
N = 512
D = 512
SCALE = DH ** -0.5
N_CORES = 8
QK_SCALE = 16.0  # q/k pre-scale into fp8 range; folded out in exp scale

F32 = mybir.dt.float32
F16 = mybir.dt.float16
FP8 = mybir.dt.float8e4
DR = mybir.MatmulPerfMode.DoubleRow
NP_F16 = np.float16
NP_FP8 = ml_dtypes.float8_e4m3


def qk_pass_channels(base):
    """Channel order for the 4 folded QK projection passes (g, j).

    Pass (g, j) covers heads 4g..4g+3; PSUM partition p holds channel
    base + head(g,p)*64 + (p%32) + 32*j, so the SBUF tile [128, 2, n]
    (partition p, free j) is exactly the DR lhsT/rhs fold (dh = p%32+32j,
    head = p//32 within group).
    """
    cols = []
    for g in range(2):
        for j in range(2):
            for p in range(128):
                head = 4 * g + p // 32
                dh = (p % 32) + 32 * j
                cols.append(base + head * 64 + dh)
    return cols


def w_perm_cols():
    """Column permutation for the device Wqkv tensor: 4 K passes, 4 Q
    passes (128 cols each), then V natural."""
    return qk_pass_channels(DI) + qk_pass_channels(0) + list(range(2 * DI, 3 * DI))


W_PERM = np.array(w_perm_cols())


def _emit_body(nc, tc, dio, pools, n_c):
    (p_w, p_x, p_xq, p_kq, p_v, p_ex, p_oT, p_wo, p_ob, p_oc,
     p_den, psA, psB) = pools
    Exp = mybir.ActivationFunctionType.Exp
    mm = nc.tensor.matmul

    w, xt, xqt, wo = {}, {}, {}, {}
    kF, qF, V, oT = {}, {}, {}, {}

    xt8, wk8 = {}, {}

    def alloc_inputs(i):
        w[i] = p_w.tile([128, 4, 1536], F16, tag=f"w{i}", name=f"w{i}")
        xt[i] = p_x.tile([128, 4, 512], F16, tag=f"xt{i}", name=f"xt{i}")
        xt8[i] = p_x.tile([128, 4, 512], FP8, tag=f"xt8{i}", name=f"xt8{i}")
        wk8[i] = p_x.tile([128, 4, 512], FP8, tag=f"wk8{i}", name=f"wk8{i}")
        xqt[i] = p_xq.tile([128, 4, n_c], F16, tag=f"xqt{i}", name=f"xqt{i}")

    # one DMA instruction per logical block: each instruction costs a fixed
    # ~625 ns on the serial HWDGE device, so instruction count is what matters
    def dma_w(i, c0, c1):
        src = dio[f"W{i}"].rearrange("(a p) c -> p a c", p=128)
        nc.sync.dma_start(w[i][:, :, c0:c1], src[:, :, c0:c1])

    def dma_xt(i):
        src = dio[f"xT{i}"].rearrange("(a p) c -> p a c", p=128)
        nc.sync.dma_start(xt[i][:, :, :], src)

    def dma_xt8(i):
        src8 = dio[f"xT8{i}"].rearrange("(a p) c -> p a c", p=128)
        nc.sync.dma_start(xt8[i][:, :, :], src8)

    def dma_wk8(i, c0, c1):
        srcw = dio[f"Wk8{i}"].rearrange("(a p) c -> p a c", p=128)
        nc.sync.dma_start(wk8[i][:, :, c0:c1], srcw[:, :, c0:c1])

    def dma_xqt(i):
        src = dio[f"xqT{i}"].rearrange("(a p) c -> p a c", p=128)
        nc.sync.dma_start(xqt[i][:, :, :], src)

    def load_wo(i):
        wo[i] = p_wo.tile([128, 4, 512], F16, tag="wo", name=f"wo{i}")
        src = dio[f"Wout{i}"].rearrange("(a p) c -> p a c", p=128)
        nc.sync.dma_start(wo[i][:, :, :], src)

    def alloc_mod(i):
        kF[i] = [
            p_kq.tile([128, 2, 512], FP8, tag=f"kF{i}{g}", name=f"kF{i}{g}")
            for g in range(2)
        ]
        qF[i] = [
            p_kq.tile([128, 2, n_c], FP8, tag=f"qF{i}{g}", name=f"qF{i}{g}")
            for g in range(2)
        ]
        V[i] = p_v.tile([128, 4, 8, 65], F16, tag=f"V{i}", name=f"V{i}")
        nc.vector.memset(V[i][:, :, :, 64:65], 1.0)
        oT[i] = p_oT.tile([128, 4, n_c], F16, tag=f"oT{i}", name=f"oT{i}")

    def proj_ps(kind, name):
        # "A": prologue passes double-buffer through the idle psA pool;
        # "B": the psB "mm" slot; "C": the psB "avo" slot (free until the
        # first AV accumulation opens) -- B/C alternation double-buffers
        # the start-region passes.
        if kind == "A":
            t = psA.tile([128, 3, 512], F32, tag="dots", name=name)
            return t[:, 0, :]
        tag = "avo" if kind == "C" else "mm"
        return psB.tile([128, 512], F32, tag=tag, name=name)

    def k_pass(i, g, j, pk="B"):
        # fp8 DoubleRow: contraction over d pairs kt-chunks; Wk8 is sent
        # pre-scaled x32 so kF = 0.5*psum carries the same x16 as qF.
        # Split into column halves so the first psum->SBUF copy overlaps
        # the second half's matmuls (the "mm" slot is single-buffered).
        ps = proj_ps(pk, f"psk{i}{g}{j}")
        wsl = wk8[i][:, :, (2 * g + j) * 128 : (2 * g + j + 1) * 128]
        for half in range(2):
            sl = slice(half * 256, (half + 1) * 256)
            for kp in range(2):
                mm(ps[:, sl],
                   wsl[:, 2 * kp : 2 * kp + 2, :],
                   xt8[i][:, 2 * kp : 2 * kp + 2, sl],
                   start=kp == 0, stop=kp == 1, perf_mode=DR)
            nc.vector.tensor_scalar_mul(kF[i][g][:, j, sl], ps[:, sl], 0.5)

    def q_pass(i, g, j, pk="B", act_copy=False):
        # act_copy: route the odd half's psum->fp8 copy through ScalarE
        # (idle in the prologue) so DVE isn't the pass serializer.
        ps = proj_ps(pk, f"psq{i}{g}{j}")
        wsl = w[i][:, :, 512 + (2 * g + j) * 128 : 512 + (2 * g + j + 1) * 128]
        h1 = n_c // 2
        for half in range(2):
            sl = slice(half * h1, (half + 1) * h1 if half == 0 else n_c)
            for kt in range(4):
                mm(ps[:, sl], wsl[:, kt, :], xqt[i][:, kt, sl],
                   start=kt == 0, stop=kt == 3)
            if act_copy and half == 1:
                nc.scalar.mul(qF[i][g][:, j, sl], ps[:, sl], QK_SCALE)
            else:
                nc.vector.tensor_scalar_mul(qF[i][g][:, j, sl], ps[:, sl], QK_SCALE)

    def v_pass(i, mch, pk="B"):
        ps = proj_ps(pk, f"psv{i}{mch}")
        for half in range(2):
            sl = slice(half * 256, (half + 1) * 256)
            for kt in range(4):
                mm(ps[:, sl], xt[i][:, kt, mch * 128 : (mch + 1) * 128],
                   w[i][:, kt, 1024 + half * 256 : 1280 + half * 256],
                   start=kt == 0, stop=kt == 3)
            nc.vector.tensor_copy(
                V[i][:, mch, half * 4 : (half + 1) * 4, 0:64],
                ps[:, sl].rearrange("p (h x) -> p h x", x=64),
            )

    # Attention is split into a dots/exp part and a deferred AV part: the
    # dots of head h+1 are emitted before the AV of head h, so the in-order
    # PE keeps ScalarE fed while the AV accumulator slot ("avo", single
    # buffered) drains through DVE/Pool.
    chunks = [(j_, c) for j_ in range(3) for c in range(4)]
    pending_ex = {}
    pending_pso = {}

    def head_dots(i, h, between=None):
        g, hh = h // 4, h % 4
        kslc = lambda j_, c: kF[j_][g][hh * 32 : hh * 32 + 32, :, c * 128 : (c + 1) * 128]
        qslc = qF[i][g][hh * 32 : hh * 32 + 32, :, :]
        exs = []
        pending_ex[(i, h)] = exs  # filled in place; av parts may read early
        for t in range(4):
            ps_d = psA.tile([128, 3, 512], F32, tag="dots", name=f"psd{i}{h}{t}")
            for u in range(3):
                j_, c = chunks[t * 3 + u]
                mm(ps_d[:, u, 0:n_c], kslc(j_, c), qslc,
                   start=True, stop=True, perf_mode=DR,
                   tile_position=(hh * 32, 0))
            if between is not None and t < len(between) and between[t] is not None:
                for fn in between[t]:
                    fn()
            ex = p_ex.tile([128, 3, n_c], F16, tag="ex", name=f"ex{i}{h}{t}")
            nc.scalar.activation(
                ex[:, :, :], ps_d[:, :, 0:n_c], Exp, scale=SCALE / (QK_SCALE**2)
            )
            exs.append(ex)

    def av_part(i, h, j_):
        # one modality's worth (4 chunks) of the AV accumulation
        if j_ == 0:
            pending_pso[(i, h)] = psB.tile(
                [128, 512], F32, tag="avo", name=f"pso{i}{h}"
            )
        ps_o = pending_pso[(i, h)]
        exs = pending_ex[(i, h)]
        for c in range(4):
            t, u = divmod(j_ * 4 + c, 3)
            mm(ps_o[0:65, 0:n_c], V[j_][:, c, h, :], exs[t][:, u, :],
               start=j_ == 0 and c == 0, stop=j_ == 2 and c == 3)

    def av_drain(i, h):
        # copy psum -> SBUF f16 first: frees the single "avo" psum slot after
        # ~300ns instead of holding it through the recip/broadcast/mul chain
        ps_o = pending_pso.pop((i, h))
        pending_ex.pop((i, h))
        oc = p_oc.tile([65, 512], F16, tag="oc", name=f"oc{i}{h}")
        nc.vector.tensor_copy(oc[:, 0:n_c], ps_o[0:65, 0:n_c])
        den_s = p_den.tile([1, 512], F32, tag="dens", name=f"ds{i}{h}")
        nc.vector.reciprocal(den_s[0:1, 0:n_c], oc[64:65, 0:n_c])
        den_r = p_den.tile([64, 512], F32, tag="denr", name=f"dr{i}{h}")
        nc.gpsimd.partition_broadcast(den_r[:, 0:n_c], den_s[0:1, 0:n_c])
        po, fh = (h % 2) * 64, h // 2
        nc.vector.tensor_mul(
            oT[i][po : po + 64, fh, :], oc[0:64, 0:n_c], den_r[:, 0:n_c]
        )

    def head_av(i, h):
        for j_ in range(3):
            av_part(i, h, j_)
        av_drain(i, h)

    ob_t = {}

    def outproj_chunk(i, s, nn, pk="B"):
        # f16 output staged into one [128, 3, 512] tile; chunks 0+1 go out in
        # a single DMA (dram padded to 384 rows), the 19-row tail separately.
        if s == 0:
            ob_t[i] = p_ob.tile([128, 3, 512], F16, tag="ob", name=f"ob{i}")
        ps = proj_ps(pk, f"psf{i}{s}")
        ci = s // 128
        for half in range(2):
            sl = slice(half * 256, (half + 1) * 256)
            for ct in range(4):
                mm(ps[0:nn, sl], oT[i][:, ct, s : s + nn], wo[i][:, ct, sl],
                   start=ct == 0, stop=ct == 3)
            nc.vector.tensor_copy(ob_t[i][0:nn, ci, sl], ps[0:nn, sl])
        dst = dio[f"out{i}"].rearrange("(c p) d -> p c d", p=128)
        if ci == 1:
            nc.sync.dma_start(dst[:, 0:2, :], ob_t[i][:, 0:2, :])
        elif ci == 2:
            nc.sync.dma_start(dio[f"out{i}"][256 : 256 + nn, :], ob_t[i][0:nn, 2, :])

    # ---------------- schedule ----------------
    # DMA emission order = need order: K-g0 cols of all mods first (the
    # prologue), then Q0-g0/V0, V1/V2 (extras of head (0,0)), K-g1
    # (extras (0,1)-(0,3)), then the rest.
    for i in range(3):
        alloc_inputs(i)
    dma_wk8(0, 0, 256)   # K g0 operands (fp8 DR) for the prologue
    dma_xt8(0)
    dma_w(0, 512, 768)   # Q0 g0
    dma_xqt(0)
    dma_wk8(1, 0, 256)
    dma_xt8(1)
    dma_wk8(2, 0, 256)
    dma_xt8(2)
    dma_xt(0)            # V operands (f16 x + W V cols)
    dma_w(0, 1024, 1536)
    dma_xt(1)
    dma_w(1, 1024, 1536)
    dma_xt(2)
    dma_w(2, 1024, 1536)
    dma_w(1, 512, 1024)  # Q1 (needed at the (0,3)->(1,0) boundary)
    dma_xqt(1)
    for i in range(3):   # K g1
        dma_wk8(i, 256, 512)
    dma_w(0, 768, 1024)
    dma_w(2, 512, 1024)
    dma_xqt(2)
    load_wo(0)
    load_wo(1)

    # PE warm-up: dependency-free matmuls burn the p-state ramp while the
    # first DMAs land, so the projection passes run at full clock.  Emitted
    # before the V memsets so the warm tile's memset is DVE's first op and
    # the ramp starts ~1.2us in; 38 reps end right as the K operands land.
    warm = p_den.tile([128, 256], F16, tag="warm", name="warm")
    nc.vector.memset(warm[:, :], 0.0)
    ps_warm = psB.tile([128, 512], F32, tag="mm", name="ps_warm")
    for r in range(38):
        mm(ps_warm[:, 0:64], warm[:, 0:128], warm[:, 0:64],
           start=True, stop=True)

    for i in range(3):
        alloc_mod(i)

    # ------------- slot-pipelined main schedule -------------
    # The exp chain is the backbone: each window (i, h) emits its 4 dots
    # groups with all other PE work (pending heads' AV + projection passes +
    # out-projections) split across the 4 between-slots, so the next dots
    # group is always near the head of PE's in-order queue when its psA
    # buffer frees.  Filler is hand-placed per window; ~2.8us capacity each
    # (4 exp slots of 873ns minus 684ns of dots).
    for j in range(2):
        k_pass(0, 0, j, pk="A")
    for j in range(2):
        q_pass(0, 0, j, pk="A")

    nch_list = [(s, min(128, n_c - s)) for s in range(0, n_c, 128)]
    AB = ("B", "C")  # pre-AV psB double-buffer alternation
    BB = ("B", "B")  # post-AV: "C" (avo) is owned by AV accumulation

    def kp(i, g, pks=BB):
        return [(212, lambda j=j, p=pks[j]: k_pass(i, g, j, pk=p))
                for j in range(2)]

    def qp(i, g, pks=BB):
        return [(458, lambda j=j, p=pks[j]: q_pass(i, g, j, pk=p))
                for j in range(2)]

    def vp(i, c, pk):
        return [(856, lambda: v_pass(i, c, pk=pk))]

    def av4(i, h):
        its = [(460, lambda j_=j_: av_part(i, h, j_)) for j_ in range(3)]
        its.append((100, lambda: av_drain(i, h)))
        return its

    def op(i, k):
        s, nn = nch_list[k]
        return [(856, lambda: outproj_chunk(i, s, nn))]

    def zipav(av, other):
        out, a, b = [], list(av), list(other)
        while a or b:
            if b:
                out.append(b.pop(0))
            if a:
                out.append(a.pop(0))
        return out

    def split4(items):
        slots = [[], [], [], []]
        total = sum(c for c, _ in items)
        cum = 0
        for c, fn in items:
            slots[min(3, 4 * cum // max(total, 1))].append(fn)
            cum += c
        return slots

    # AV of head X drains in a later window (lag 1-5; ex pool holds 6 heads).
    # Projection placement: every dependency lands >= 1 window early; K g1 of
    # all mods done by (2,2) < first g1 window (0,4); Q(i)g1 done 4+ windows
    # before (i,4); outproj(i) right after drain(i,7).
    W = {
        (0, 0): kp(1, 0, AB) + kp(2, 0, AB),
        (0, 1): vp(0, 0, "B") + vp(0, 1, "C") + vp(0, 2, "B"),
        (0, 2): vp(0, 3, "C") + vp(1, 0, "B") + vp(1, 1, "C"),
        (0, 3): vp(1, 2, "B") + vp(1, 3, "C") + qp(1, 0, AB),
        (1, 0): vp(2, 0, "B") + vp(2, 1, "C") + vp(2, 2, "B"),
        (1, 1): vp(2, 3, "C") + av4(0, 0),
        (1, 2): zipav(av4(0, 1), qp(2, 0)),
        (1, 3): av4(0, 2) + av4(0, 3),
        (2, 0): zipav(av4(1, 0), kp(0, 1)),
        (2, 1): zipav(av4(1, 1), kp(1, 1)),
        (2, 2): zipav(av4(1, 2), kp(2, 1)),
        (2, 3): zipav(av4(1, 3), qp(0, 1)),
        (0, 4): zipav(av4(2, 0), qp(1, 1)),
        (0, 5): zipav(av4(2, 1), qp(2, 1)),
        (0, 6): av4(2, 2) + av4(2, 3),
        (0, 7): av4(0, 4) + [(50, lambda: load_wo(2))],
        (1, 4): av4(0, 5) + av4(0, 6),
        (1, 5): av4(0, 7) + op(0, 0),
        (1, 6): op(0, 1) + op(0, 2),
        (1, 7): av4(1, 4) + av4(1, 5),
        (2, 4): av4(1, 6) + av4(1, 7),
        (2, 5): zipav(av4(2, 4), op(1, 0) + op(1, 1)),
        (2, 6): zipav(av4(2, 5), op(1, 2)),
    }
    win_seq = [
        (i, h)
        for g_ in range(2)
        for i in range(3)
        for h in range(4 * g_, 4 * g_ + 4)
    ]
    for win in win_seq[:-1]:
        head_dots(*win, between=split4(W[win]))

    # Last window: AV of (2,6) in the early slots, (2,7)'s own AV parts as
    # soon as their exp groups are emitted, so the post-exp tail is just the
    # final 4 AV matmuls + drain + out-proj of mod 2.
    head_dots(2, 7, between=[
        [fn for _, fn in av4(2, 6)[:2]],
        [fn for _, fn in av4(2, 6)[2:]],
        [lambda: av_part(2, 7, 0)],
        [lambda: av_part(2, 7, 1)],
    ])
    av_part(2, 7, 2)
    # direct drain for the last head (no avo contention left): skips the
    # psum->SBUF copy hop on the critical path
    ps_o = pending_pso.pop((2, 7))
    pending_ex.pop((2, 7))
    den_s = p_den.tile([1, 512], F32, tag="dens", name="ds27")
    nc.vector.reciprocal(den_s[0:1, 0:n_c], ps_o[64:65, 0:n_c])
    den_r = p_den.tile([64, 512], F32, tag="denr", name="dr27")
    nc.gpsimd.partition_broadcast(den_r[:, 0:n_c], den_s[0:1, 0:n_c])
    nc.vector.tensor_mul(oT[2][64:128, 3, :], ps_o[0:64, 0:n_c], den_r[:, 0:n_c])

    # tail out-projection of mod 2: emit the ct<=2 accumulation matmuls for
    # ALL chunks first -- they only read heads 0-5 of oT (long drained), so
    # they run during the (2,6)/(2,7) drain chains; only the six ct=3
    # matmuls wait for the final muls.  3 psum regions carved from the two
    # free psA tiles.
    tA = psA.tile([128, 3, 512], F32, tag="dots", name="psfA")
    tB = psA.tile([128, 3, 512], F32, tag="dots", name="psfB")
    regions = [tA[:, 0, :], tA[:, 1, :], tB[:, 0, :]]
    ob_t[2] = p_ob.tile([128, 3, 512], F16, tag="ob", name="ob2")
    for k_, (s, nn) in enumerate(nch_list):
        ps = regions[k_]
        for half in range(2):
            sl = slice(half * 256, (half + 1) * 256)
            for ct in range(3):
                mm(ps[0:nn, sl], oT[2][:, ct, s : s + nn], wo[2][:, ct, sl],
                   start=ct == 0, stop=False)
    dst = dio["out2"].rearrange("(c p) d -> p c d", p=128)
    for k_, (s, nn) in enumerate(nch_list):
        ps = regions[k_]
        for half in range(2):
            sl = slice(half * 256, (half + 1) * 256)
            mm(ps[0:nn, sl], oT[2][:, 3, s : s + nn], wo[2][:, 3, sl],
               start=False, stop=True)
            nc.vector.tensor_copy(ob_t[2][0:nn, k_, sl], ps[0:nn, sl])
        if k_ == 1:
            nc.sync.dma_start(dst[:, 0:2, :], ob_t[2][:, 0:2, :])
        elif k_ == 2:
            nc.sync.dma_start(dio["out2"][256 : 256 + nn, :], ob_t[2][0:nn, 2, :])


def build(n_c=275, reps=1):
    assert 256 < n_c <= 384, "out-DMA chunking hardcodes the 3-chunk case"
    nc = bacc.Bacc("TRN2", target_bir_lowering=False, debug=False)
    dio = {}
    for i in range(3):
        dio[f"xT{i}"] = nc.dram_tensor(f"xT{i}", [D, N], F16, kind="ExternalInput").ap()
        dio[f"xT8{i}"] = nc.dram_tensor(
            f"xT8{i}", [D, N], FP8, kind="ExternalInput"
        ).ap()
        dio[f"Wk8{i}"] = nc.dram_tensor(
            f"Wk8{i}", [D, DI], FP8, kind="ExternalInput"
        ).ap()
        dio[f"xqT{i}"] = nc.dram_tensor(
            f"xqT{i}", [D, n_c], F16, kind="ExternalInput"
        ).ap()
        dio[f"W{i}"] = nc.dram_tensor(
            f"W{i}", [D, 3 * DI], F16, kind="ExternalInput"
        ).ap()
        dio[f"Wout{i}"] = nc.dram_tensor(
            f"Wout{i}", [DI, D], F16, kind="ExternalInput"
        ).ap()
        dio[f"out{i}"] = nc.dram_tensor(
            f"out{i}", [384, D], F16, kind="ExternalOutput"
        ).ap()
    with tile.TileContext(nc) as tc:
        with (
            tc.tile_pool(name="w", bufs=1) as p_w,
            tc.tile_pool(name="x", bufs=1) as p_x,
            tc.tile_pool(name="xq", bufs=1) as p_xq,
            tc.tile_pool(name="kq", bufs=1) as p_kq,
            tc.tile_pool(name="v", bufs=1) as p_v,
            tc.tile_pool(name="ex", bufs=24) as p_ex,
            tc.tile_pool(name="oT", bufs=1) as p_oT,
            tc.tile_pool(name="wo", bufs=3) as p_wo,
            tc.tile_pool(name="ob", bufs=2) as p_ob,
            tc.tile_pool(name="oc", bufs=3) as p_oc,
            tc.tile_pool(name="den", bufs=4) as p_den,
            tc.tile_pool(name="psA", bufs=2, space="PSUM") as psA,
            tc.tile_pool(name="psB", bufs=1, space="PSUM") as psB,
        ):
            pools = (
                p_w, p_x, p_xq, p_kq, p_v, p_ex, p_oT, p_wo, p_ob, p_oc,
                p_den, psA, psB,
            )
            for _ in range(reps):
                _emit_body(nc, tc, dio, pools, n_c)
    nc.compile()
    return nc


_BUILD_CACHE = {}


def _get_built(n_c, reps):
    key = (n_c, reps)
    if key not in _BUILD_CACHE:
        _BUILD_CACHE[key] = build(n_c, reps)
    return _BUILD_CACHE[key]


def pick_n_c(inputs):
    """Smallest compacted-query count: unmasked rows + 1 dummy zero row."""
    need = 0
    for i in range(3):
        m = np.asarray(inputs[f"m{i}"]).astype(bool)
        for b in range(B):
            n_u = int(m[b].sum())
            need = max(need, n_u + (1 if n_u < N else 0))
    return max(need, 192)


WK8_SCALE = 32.0  # Wk pre-scale into fp8 range; kF copy multiplies by
                  # QK_SCALE/WK8_SCALE so kF carries the same x16 as qF


def make_in_maps(inputs, n_c=275):
    """Per-core input dicts: transposed fp16 x (+fp8 copy for the K
    DoubleRow projection), compacted fp16 xq, column-permuted fp16 Wqkv,
    fp8 x32 K-weights, fp16 Wout."""
    xs = [np.asarray(inputs[f"x{i}"], dtype=np.float32) for i in range(3)]
    ms = [np.asarray(inputs[f"m{i}"]).astype(bool) for i in range(3)]
    Wp = [
        np.asarray(inputs[f"Wqkv{i}"], np.float32)[:, W_PERM].astype(NP_F16)
        for i in range(3)
    ]
    Wk8 = [
        np.ascontiguousarray(
            WK8_SCALE
            * np.asarray(inputs[f"Wqkv{i}"], np.float32)[:, W_PERM[0:DI]]
        ).astype(NP_FP8)
        for i in range(3)
    ]
    Wo = [np.asarray(inputs[f"Wout{i}"], np.float32).astype(NP_F16) for i in range(3)]
    in_maps = []
    for b in range(B):
        m = {}
        for i in range(3):
            xb = xs[i][b]
            xbT = np.ascontiguousarray(xb.T)
            m[f"xT{i}"] = xbT.astype(NP_F16)
            m[f"xT8{i}"] = xbT.astype(NP_FP8)
            sel = np.flatnonzero(ms[i][b])
            xq = np.zeros((n_c, D), np.float32)
            xq[: len(sel)] = xb[sel[:n_c]]
            m[f"xqT{i}"] = np.ascontiguousarray(xq.T.astype(NP_F16))
            m[f"W{i}"] = Wp[i]
            m[f"Wk8{i}"] = Wk8[i]
            m[f"Wout{i}"] = Wo[i]
        in_maps.append(m)
    return in_maps


def scatter_outputs(results, inputs, n_c):
    ms = [np.asarray(inputs[f"m{i}"]).astype(bool) for i in range(3)]
    outs = []
    for i in range(3):
        full = np.empty((B, N, D), np.float32)
        for b in range(B):
            comp = np.asarray(results[b][f"out{i}"], np.float32)
            sel = np.flatnonzero(ms[i][b])
            full[b][sel] = comp[: len(sel)]
            if len(sel) < N:
                full[b][~ms[i][b]] = comp[len(sel)]
        outs.append(full)
    return outs


def kernel(**inputs):
    n_c = pick_n_c(inputs)
    in_maps = make_in_maps(inputs, n_c)
    nc = _get_built(n_c=n_c, reps=1)
    res = bass_utils.run_bass_kernel_spmd(nc, in_maps, core_ids=list(range(N_CORES)))
    return tuple(scatter_outputs(res.results, inputs, n_c))

